# revision 1
# baseline (speedup 1.0000x reference)
"""Trainium2 Bass kernel for nn_ExampleModel_1116691497724 (moe_routing).

Math: the reference returns log_softmax_T( sum_D(moe_out) ), and sum_D
collapses the expert FFN to a dot product:
    sum_d (h @ W2[e] + b2[e]) = h . w2sum[e] + sum(b2[e]),  w2sum[e] = W2[e] @ 1
    (x @ W1[e] + b1[e]) . w2sum[e] = x . v[e] + c[e]
with v[e] = W1[e] @ w2sum[e]  (a [D] vector) and scalar
c[e] = b1[e].w2sum[e] + sum(b2[e]).  Then per token:
    s_e = x . v[e] + c[e],  logits = x @ Wg
    moe_sum = max(softmax(logits)) * s_argmax(logits)
    out = log_softmax over tokens (per batch row) of moe_sum.

Distribution over 8 cores, two launches (measured: a single ncfw collective
costs ~65us of barrier/trigger latency on this runtime — far more than a
second launch's fixed ~17us, so the 16KB cross-core combine happens on the
host between launches; the host does only that partial sum, all real math
stays on device):
  launch A (expert-parallel over H): core c reduces W2[:, 128c:128c+128, :]
    and computes partial v from the matching W1 columns (f32r stream after a
    rounding pass) -> outputs [v0 | v1 | c0 c1] partials (16KB); host sums.
  launch B (token-parallel): core c owns batch row c%4 (512 tokens): logits
    stream in fp32 (exact — argmax ties must match the reference), s stream
    in f32r, gate/select per token after a PE transpose, row log_softmax via
    PE transposes (no cross-partition DMA).  Host takes rows from cores 0..3.

Scheduling: stationary matmul operands are tiny (M<=4) so LDWEIGHTS is
negligible; fp32 streams at 4 cycles/row, f32r at 1.  Big loads alternate the
two HWDGE rings (SP via nc.sync, ACT via nc.scalar) for concurrency.  The d
axis is decomposed as d = p*16 + n so the flat v vector loads into [128,16]
tiles with contiguous per-partition runs.
"""

import sys

import numpy as np

for _p in ("/opt/trn_rl_repo",):
    if _p not in sys.path:
        sys.path.append(_p)

import concourse.bass as bass  # noqa: E402
import concourse.mybir as mybir  # noqa: E402
import concourse.tile as tile  # noqa: E402
from concourse import bacc, bass_utils  # noqa: E402
from concourse.masks import make_identity  # noqa: E402

# Problem shape (hardcoded per spec).
B, T, D, H, E = 4, 512, 2048, 1024, 2
P = 128
NCORES = 8
TB = T  # tokens per core = one batch row
NB = D // P  # 16 d-blocks
HC = H // NCORES  # 128 h-chunk per expert per core
NG = TB // P  # 4 token groups per core
DC = D // NCORES  # 256 b2 columns per core
VK = 4  # v computed in VK chunks of D/VK columns
F32 = mybir.dt.float32
F32R = mybir.dt.float32r
AX = mybir.AxisListType
AF = mybir.ActivationFunctionType
ALU = mybir.AluOpType

VPART = 2 * D + 2  # launch A output: v0 | v1 | c0 c1
BF16 = mybir.dt.bfloat16
BF16_W = False  # bf16 W1/W2 saves only ~2us but costs 13x accuracy; keep f32


def emit_phase_a(nc, tc, io):
    """w2sum + partial v for this core's H-chunk -> vpart [1, 2D+2]."""
    w1t, w2r, b1c, b2c, vout = io["w1t"], io["w2r"], io["b1c"], io["b2c"], io["vout"]
    with (
        tc.tile_pool(name="main", bufs=1) as pool,
        tc.tile_pool(name="psum", bufs=1, space="PSUM") as psum,
    ):
        # DMA plan: tiny contiguous bias rows FIRST on the sync ring (so no
        # DVE op ever head-of-line blocks on them), then W2 halves (they gate
        # the reduce), then W1 split over all three queues.  W1 goes straight
        # into an f32r tile (w1t is declared float32r) — no cast pass.
        HD = D // 2
        WDT = BF16 if BF16_W else F32
        VDT = BF16 if BF16_W else F32R
        b1_sb = pool.tile([1, E * HC], F32)
        nc.sync.dma_start(b1_sb[:], b1c)
        b2_sb = pool.tile([1, E * DC], F32)
        nc.sync.dma_start(b2_sb[:], b2c)
        w2_sb = pool.tile([P, E, D], WDT)
        w1r = pool.tile([P, E, D], VDT)
        for h in range(2):
            nc.sync.dma_start(w2_sb[:, 0, h * HD : (h + 1) * HD], w2r[0, :, h * HD : (h + 1) * HD])
            nc.scalar.dma_start(w2_sb[:, 1, h * HD : (h + 1) * HD], w2r[1, :, h * HD : (h + 1) * HD])
        for h in range(2):
            nc.sync.dma_start(w1r[:, 0, h * HD : (h + 1) * HD], w1t[0, :, h * HD : (h + 1) * HD])
            nc.scalar.dma_start(w1r[:, 1, h * HD : (h + 1) * HD], w1t[1, :, h * HD : (h + 1) * HD])

        # b1 row -> partition-major [128, E] via PE transpose (identity [1,1])
        one1 = pool.tile([1, 1], F32)
        nc.gpsimd.memset(one1[:], 1.0)
        b1t_ps = psum.tile([P, E], F32)
        for e in range(E):
            nc.tensor.transpose(
                b1t_ps[:, e : e + 1], b1_sb[0:1, e * HC : (e + 1) * HC], one1[:]
            )
        b1p = pool.tile([P, E], F32)
        nc.vector.tensor_copy(b1p[:], b1t_ps[:])

        w2h = pool.tile([P, 2 * E], F32)
        w2s = pool.tile([P, E], F32)
        for e in range(E):
            for h in range(2):
                nc.vector.reduce_sum(
                    w2h[:, 2 * e + h : 2 * e + h + 1],
                    w2_sb[:, e, h * HD : (h + 1) * HD],
                    axis=AX.X,
                )
            nc.vector.tensor_add(
                w2s[:, e : e + 1], w2h[:, 2 * e : 2 * e + 1], w2h[:, 2 * e + 1 : 2 * e + 2]
            )
        w2s_r = pool.tile([P, E], VDT)
        nc.vector.tensor_copy(w2s_r[:], w2s[:])
        b2s = pool.tile([1, E], F32)
        for e in range(E):
            nc.vector.reduce_sum(
                b2s[0:1, e : e + 1], b2_sb[0:1, e * DC : (e + 1) * DC], axis=AX.X
            )

        pay = pool.tile([1, VPART], F32)
        b1dot = psum.tile([1, E], F32)
        DK = D // VK
        for e in range(E):
            for k in range(VK):
                vch = psum.tile([1, DK], F32, name="vch", tag="vch", bufs=2)
                nc.tensor.matmul(
                    vch[:],
                    w2s_r[:, e : e + 1],
                    w1r[:, e, k * DK : (k + 1) * DK],
                    start=True,
                    stop=True,
                )
                dst = pay[0:1, e * D + k * DK : e * D + (k + 1) * DK]
                if k % 2 == 0:
                    nc.vector.tensor_copy(dst, vch[:])
                else:
                    nc.scalar.copy(dst, vch[:])
            nc.tensor.matmul(
                b1dot[0:1, e : e + 1],
                w2s[:, e : e + 1],
                b1p[:, e : e + 1],
                start=True,
                stop=True,
            )
            nc.vector.tensor_add(
                pay[0:1, 2 * D + e : 2 * D + e + 1],
                b1dot[0:1, e : e + 1],
                b2s[0:1, e : e + 1],
            )
        nc.sync.dma_start(vout[:], pay[:])


def emit_phase_b(nc, tc, io):
    """logits (fp32) + s (f32r) streams, gate/select, row log_softmax."""
    xt, wgt, vin, out = io["xt"], io["wgt"], io["vin"], io["out"]
    rings = [nc.sync, nc.scalar]
    with (
        tc.tile_pool(name="main", bufs=1) as pool,
        tc.tile_pool(name="psum", bufs=1, space="PSUM") as psum,
    ):
        # v and Wg arrive as [16, 128] n-major rows (contiguous 512B per
        # partition = few fast packets; a partition-major load would emit 64B
        # packets and clog a queue for several us) and get transposed on the
        # idle PE into the [128, 16] layout the stationary operand needs
        vrow = pool.tile([16, E * P], F32)
        for e in range(E):
            nc.sync.dma_start(
                vrow[:, e * P : (e + 1) * P],
                vin[0:1, e * D : (e + 1) * D].rearrange("x (n p) -> n (x p)", p=P),
            )
        wgr = pool.tile([16, E * P], F32)
        for e in range(E):
            nc.scalar.dma_start(wgr[:, e * P : (e + 1) * P], wgt[e])
        csum = pool.tile([1, E], F32)
        nc.gpsimd.dma_start(csum[:], vin[0:1, 2 * D : 2 * D + E])

        x_sb = pool.tile([P, NB, TB], F32)
        xv = xt.rearrange("(n p) t -> p n t", p=P)  # d = n*128 + p
        qs = [nc.sync, nc.scalar]
        chunks = [
            (0, 0, 1), (1, 1, 2),
            (0, 2, 4), (1, 4, 6),
            (0, 6, 9), (1, 9, 12),
            (0, 12, 14), (1, 14, 16),
        ]
        for q, lo, hi in chunks:
            qs[q].dma_start(x_sb[:, lo:hi, :], xv[:, lo:hi, :])

        # preload ACT tables (Exp, Ln) off the critical path; keep ALL copy
        # work off the scalar engine so these tables are never evicted
        warm = pool.tile([1, 2], F32)
        nc.gpsimd.memset(warm[:], 1.0)
        wz = pool.tile([1, 2], F32)
        nc.scalar.activation(wz[:], warm[:], AF.Exp)
        nc.scalar.activation(wz[:], warm[:], AF.Ln)

        ident = pool.tile([P, P], F32)
        make_identity(nc, ident[:])
        # m4[p, n, :] = [wg0 wg1 v0 v1] for d-block n (d = n*128 + p): one
        # M=4 fp32 stream computes logits AND s together (fp32 matmul cost is
        # per streamed row, independent of stationary columns)
        m4 = pool.tile([P, NB, 4], F32)
        for e in range(E):
            wtp = psum.tile([P, NB], F32, name=f"wtp_{e}", tag="tp16", bufs=2)
            nc.tensor.transpose(wtp[:], wgr[:, e * P : (e + 1) * P], ident[0:16, 0:16])
            nc.vector.tensor_copy(m4[:, :, e : e + 1], wtp[:, :, None])
            vtp = psum.tile([P, NB], F32, name=f"vtp_{e}", tag="tp16", bufs=2)
            nc.tensor.transpose(vtp[:], vrow[:, e * P : (e + 1) * P], ident[0:16, 0:16])
            nc.vector.tensor_copy(m4[:, :, 2 + e : 3 + e], vtp[:, :, None])
        # c broadcast tile: [0, 0, c0, c1] on every partition
        cb4 = pool.tile([P, 4], F32)
        nc.gpsimd.memset(cb4[:, 0:2], 0.0)
        nc.gpsimd.partition_broadcast(cb4[:, 2:4], csum[0:1, :])

        ps4 = psum.tile([4, TB], F32)
        for n in range(NB):
            nc.tensor.matmul(
                ps4[:], m4[:, n, :], x_sb[:, n, :], start=(n == 0), stop=(n == NB - 1)
            )
        sbl = pool.tile([4, TB], F32)
        nc.vector.tensor_copy(sbl[:], ps4[:])

        moe_sb = pool.tile([P, NG], F32)
        for g in range(NG):
            tpl = psum.tile([P, 4], F32, name=f"tpl_{g}", tag="tp", bufs=2)
            nc.tensor.transpose(tpl[:], sbl[0:4, g * P : (g + 1) * P], ident[0:4, 0:4])
            t4 = pool.tile([P, 4], F32, name=f"t4_{g}")
            nc.vector.tensor_add(t4[:], tpl[:], cb4[:])  # adds c to the s cols
            negm = pool.tile([P, 1], F32, name=f"negm_{g}")
            nc.vector.reduce_max(negm[:], t4[:, 0:2], axis=AX.X, negate=True)
            z = pool.tile([P, E], F32, name=f"z_{g}")
            den = pool.tile([P, 1], F32, name=f"den_{g}")
            nc.scalar.activation(z[:], t4[:, 0:2], AF.Exp, bias=negm[:], accum_out=den[:])
            rec = pool.tile([P, 1], F32, name=f"rec_{g}")
            nc.vector.reciprocal(rec[:], den[:])
            zmax = pool.tile([P, 1], F32, name=f"zmax_{g}")
            nc.vector.reduce_max(zmax[:], z[:], axis=AX.X)
            gate = pool.tile([P, 1], F32, name=f"gate_{g}")
            nc.vector.tensor_mul(gate[:], zmax[:], rec[:])
            mask = pool.tile([P, 1], F32, name=f"mask_{g}")
            nc.vector.tensor_tensor(mask[:], t4[:, 0:1], t4[:, 1:2], op=ALU.is_ge)
            sdiff = pool.tile([P, 1], F32, name=f"sdiff_{g}")
            nc.vector.tensor_sub(sdiff[:], t4[:, 2:3], t4[:, 3:4])
            ssel = pool.tile([P, 1], F32, name=f"ssel_{g}")
            nc.vector.tensor_mul(ssel[:], mask[:], sdiff[:])
            nc.vector.tensor_add(ssel[:], ssel[:], t4[:, 3:4])
            nc.vector.tensor_mul(moe_sb[:, g : g + 1], gate[:], ssel[:])

        # row log_softmax over all 512 tokens, via PE transposes
        tp4 = psum.tile([NG, P], F32)
        nc.tensor.transpose(tp4[:], moe_sb[:], ident[:])
        sb4t = pool.tile([NG, P], F32)
        nc.vector.tensor_copy(sb4t[:], tp4[:])
        m4p = pool.tile([NG, 1], F32)
        nc.vector.reduce_max(m4p[:], sb4t[:], axis=AX.X)
        m1p = psum.tile([1, NG], F32, name="m1p", tag="t1", bufs=2)
        nc.tensor.transpose(m1p[:], m4p[:], ident[0:NG, 0:NG])
        negm2 = pool.tile([1, 1], F32)
        nc.vector.reduce_max(negm2[:], m1p[:], axis=AX.X, negate=True)
        negm4 = pool.tile([NG, 1], F32)
        nc.gpsimd.partition_broadcast(negm4[:], negm2[:])
        e4 = pool.tile([NG, P], F32)
        s4 = pool.tile([NG, 1], F32)
        nc.scalar.activation(e4[:], sb4t[:], AF.Exp, bias=negm4[:], accum_out=s4[:])
        # reload the Ln table NOW (the Exp uses above evicted it) so the real
        # Ln below table-hits; overlaps the transpose+reduce on other engines
        nc.scalar.activation(wz[:], warm[:], AF.Ln)
        s1p = psum.tile([1, NG], F32, name="s1p", tag="t1", bufs=2)
        nc.tensor.transpose(s1p[:], s4[:], ident[0:NG, 0:NG])
        ssum = pool.tile([1, 1], F32)
        nc.vector.reduce_sum(ssum[:], s1p[:], axis=AX.X)
        logs = pool.tile([1, 1], F32)
        nc.scalar.activation(logs[:], ssum[:], AF.Ln)
        shift = pool.tile([1, 1], F32)
        nc.vector.tensor_sub(shift[:], negm2[:], logs[:])
        shift4 = pool.tile([NG, 1], F32)
        nc.gpsimd.partition_broadcast(shift4[:], shift[:])
        res4 = pool.tile([NG, P], F32)
        nc.vector.tensor_scalar_add(res4[:], sb4t[:], shift4[:])
        nc.sync.dma_start(out.rearrange("x (g p) -> g (x p)", p=P), res4[:])


_CACHED = {}


def build_program(which):
    if which in _CACHED:
        return _CACHED[which]
    nc = bacc.Bacc(
        "TRN2",
        target_bir_lowering=False,
        debug=False,
        enable_asserts=False,
        num_devices=NCORES,
    )
    if which == "a":
        io = {
            "w1t": nc.dram_tensor(
                "w1t", [E, HC, D], BF16 if BF16_W else F32R, kind="ExternalInput"
            ).ap(),
            "w2r": nc.dram_tensor(
                "w2r", [E, HC, D], BF16 if BF16_W else F32, kind="ExternalInput"
            ).ap(),
            "b1c": nc.dram_tensor("b1c", [1, E * HC], F32, kind="ExternalInput").ap(),
            "b2c": nc.dram_tensor("b2c", [1, E * DC], F32, kind="ExternalInput").ap(),
            "vout": nc.dram_tensor("vout", [1, VPART], F32, kind="ExternalOutput").ap(),
        }
        emit = emit_phase_a
    else:
        io = {
            "xt": nc.dram_tensor("xt", [D, TB], F32, kind="ExternalInput").ap(),
            "wgt": nc.dram_tensor("wgt", [E, NB, P], F32, kind="ExternalInput").ap(),
            "vin": nc.dram_tensor("vin", [1, VPART], F32, kind="ExternalInput").ap(),
            "out": nc.dram_tensor("out", [1, TB], F32, kind="ExternalOutput").ap(),
        }
        emit = emit_phase_b
    with tile.TileContext(nc) as tc:
        emit(nc, tc, io)
    nc.compile()
    _CACHED[which] = nc
    return nc


def shard_inputs_a(Wg, W1, b1, W2, b2):
    if BF16_W:
        import ml_dtypes

        wdt = ml_dtypes.bfloat16
    else:
        wdt = np.float32
    W1 = np.asarray(W1, np.float32)
    b1 = np.asarray(b1, np.float32)
    W2 = np.asarray(W2, np.float32)
    b2 = np.asarray(b2, np.float32)
    in_maps = []
    for c in range(NCORES):
        hs, he = c * HC, (c + 1) * HC
        in_maps.append(
            {
                "w1t": np.ascontiguousarray(W1[:, :, hs:he].transpose(0, 2, 1).astype(wdt)),
                "w2r": np.ascontiguousarray(W2[:, hs:he, :].astype(wdt)),
                "b1c": np.ascontiguousarray(b1[:, hs:he].reshape(1, E * HC)),
                "b2c": np.ascontiguousarray(
                    b2[:, c * DC : (c + 1) * DC].reshape(1, E * DC)
                ),
            }
        )
    return in_maps


def shard_inputs_b(x, Wg, vpart_sum):
    x = np.asarray(x, np.float32).reshape(B * T, D)
    Wg = np.asarray(Wg, np.float32)
    # wgt[p, n*2+e] = Wg[p*16+n, e]  (d = p*16 + n decomposition)
    # wgt[e, n, p] = Wg[n*128+p, e]  (d = n*128 + p decomposition)
    wgt = np.ascontiguousarray(Wg.reshape(NB, P, E).transpose(2, 0, 1))
    in_maps = []
    for c in range(NCORES):
        row = c % B
        in_maps.append(
            {
                "xt": np.ascontiguousarray(x[row * TB : (row + 1) * TB, :].T),
                "wgt": wgt,
                "vin": vpart_sum,
            }
        )
    return in_maps


def run_a(in_maps, **kwargs):
    return bass_utils.run_bass_kernel_spmd(
        build_program("a"), in_maps, core_ids=list(range(NCORES)), **kwargs
    )


def run_b(in_maps, **kwargs):
    return bass_utils.run_bass_kernel_spmd(
        build_program("b"), in_maps, core_ids=list(range(NCORES)), **kwargs
    )


def kernel(x, Wg, W1, b1, W2, b2):
    res_a = run_a(shard_inputs_a(Wg, W1, b1, W2, b2))
    # cross-core combine: sum of the 8 per-core partials (the gather/reshard
    # step between the two launches; 16KB, no model math beyond the reduction)
    vpart = np.sum([res_a.results[c]["vout"] for c in range(NCORES)], axis=0)
    vpart = np.ascontiguousarray(vpart, np.float32)
    res_b = run_b(shard_inputs_b(x, Wg, vpart))
    return np.concatenate([res_b.results[b]["out"] for b in range(B)], axis=0)



# revision 6
# speedup vs baseline: 1.1241x; 1.1241x over previous
"""Trainium2 Bass kernel for nn_ExampleModel_1116691497724 (moe_routing).

Math: the reference returns log_softmax_T( sum_D(moe_out) ), and sum_D
collapses the expert FFN to a dot product:
    sum_d (h @ W2[e] + b2[e]) = h . w2sum[e] + sum(b2[e]),  w2sum[e] = W2[e] @ 1
    (x @ W1[e] + b1[e]) . w2sum[e] = x . v[e] + c[e]
with v[e] = W1[e] @ w2sum[e]  (a [D] vector) and scalar
c[e] = b1[e].w2sum[e] + sum(b2[e]).  Then per token:
    s_e = x . v[e] + c[e],  logits = x @ Wg
    moe_sum = max(softmax(logits)) * s_argmax(logits)
    out = log_softmax over tokens (per batch row) of moe_sum.

Distribution over 8 cores, two launches (an on-device ncfw collective costs
~65us of barrier/trigger latency on this runtime, far more than a second
launch; the 16KB cross-core combine of v-partials happens on the host between
launches — the host does only that partial sum, all real math stays on device):
  launch A (expert-parallel over H): core c owns h-chunk [128c,128c+128) of
    both experts.  W2 ships bf16 d-major so w2sum is a PE ones-matmul
    (stationary [128d,128h] tiles, FWL bf16 loads), W1 ships bf16 h-major so
    v = w2sum^T-stationary @ W1-moving streams at 1 cyc/row.  Outputs
    [v0 | v1 | c0 c1] partials (16KB); host sums the 8 payloads.
  launch B (token-parallel): core c owns batch row c%4 (512 tokens).  x ships
    as a bf16 hi/lo pair (x = xh + xl exactly to ~2^-17), and one M=8
    stationary [wgh0 wgh1 wgl0 wgl1 vh0 vh1 vl0 vl1] (bf16 hi/lo of Wg and v)
    is streamed by xh then xl at 1 cyc/row: all four cross products accumulate
    in fp32 PSUM, so logits are fp32-grade (argmax must match the reference;
    bf16-only logits would flip near-boundary tokens) while the whole PE
    stream is 4x cheaper than an fp32 x stream.  l_e = col_e+col_{2+e},
    s_e = col_{4+e}+col_{6+e}+c_e after a PE transpose to token-major; then
    gate/select per token and the row log_softmax via PE transposes exactly
    as before (no cross-partition DMA).  Host takes rows from cores 0..3.

Scheduling: both launches issue the big HBM loads on the two HWDGE rings
(SP via nc.sync, ACT via nc.scalar) as their first instructions, before any
ACT-table load can head-of-line block a ring.  All hi/lo splits, transposes
and packing happen on the host (input reformatting only).
"""

import sys

import numpy as np

for _p in ("/opt/trn_rl_repo",):
    if _p not in sys.path:
        sys.path.append(_p)

import concourse.bass as bass  # noqa: E402
import concourse.mybir as mybir  # noqa: E402
import concourse.tile as tile  # noqa: E402
from concourse import bacc, bass_utils  # noqa: E402
from concourse.masks import make_identity  # noqa: E402

# Problem shape (hardcoded per spec).
B, T, D, H, E = 4, 512, 2048, 1024, 2
P = 128
NCORES = 8
TB = T  # tokens per core = one batch row
NB = D // P  # 16 d-blocks
HC = H // NCORES  # 128 h-chunk per expert per core
NG = TB // P  # 4 token groups per core
DC = D // NCORES  # 256 b2 columns per core
VK = 4  # v computed in VK chunks of D/VK columns
F32 = mybir.dt.float32
BF16 = mybir.dt.bfloat16
AX = mybir.AxisListType
AF = mybir.ActivationFunctionType
ALU = mybir.AluOpType

VPART = 2 * D + 2  # launch A output: v0 | v1 | c0 c1


def emit_phase_a(nc, tc, io):
    """w2sum (PE ones-matmul) + partial v for this core's H-chunk."""
    w2d, w1t, b1t, b2c, vout = io["w2d"], io["w1t"], io["b1t"], io["b2c"], io["vout"]
    with (
        tc.tile_pool(name="main", bufs=1) as pool,
        tc.tile_pool(name="psum", bufs=1, space="PSUM") as psum,
    ):
        # Big loads first on both HWDGE rings.  W2 (d-major) gates the
        # reduce, so it leads; W1 follows split across the rings.
        w2_sb = pool.tile([P, NB, E, HC], BF16)
        HN = NB // 2
        nc.sync.dma_start(w2_sb[:, 0:HN], w2d[:, 0:HN])
        nc.scalar.dma_start(w2_sb[:, HN:NB], w2d[:, HN:NB])
        w1_sb = pool.tile([P, E, D], BF16)
        nc.sync.dma_start(w1_sb[:, 0, :], w1t[:, 0, :])
        nc.scalar.dma_start(w1_sb[:, 1, :], w1t[:, 1, :])
        b1_sb = pool.tile([P, E], BF16)
        nc.gpsimd.dma_start(b1_sb[:], b1t)
        b2_sb = pool.tile([1, E * DC], F32)
        nc.gpsimd.dma_start(b2_sb[:], b2c)

        ones = pool.tile([P, 1], BF16)
        nc.gpsimd.memset(ones[:], 1.0)

        # w2sum[e, h] via PE: stationary [128d, 128h] tiles, moving ones.
        w2ps = [psum.tile([P, 1], F32, name=f"w2ps_{e}") for e in range(E)]
        for n in range(NB):
            for e in range(E):
                nc.tensor.matmul(
                    w2ps[e][:],
                    w2_sb[:, n, e, :],
                    ones[:],
                    start=(n == 0),
                    stop=(n == NB - 1),
                )
        # w2sum as bf16 hi+lo pair (cast error would otherwise dominate the
        # accuracy budget; both halves accumulate into the same PSUM below)
        w2s = pool.tile([P, E], BF16)
        w2sf = pool.tile([P, E], F32)
        w2sl = pool.tile([P, E], BF16)
        for e in range(E):
            nc.vector.tensor_copy(w2s[:, e : e + 1], w2ps[e][:])
            nc.vector.tensor_copy(w2sf[:, e : e + 1], w2ps[e][:])
        w2sr = pool.tile([P, E], F32)
        nc.vector.tensor_copy(w2sr[:], w2s[:])
        nc.vector.tensor_sub(w2sl[:], w2sf[:], w2sr[:])

        # v[e] = w2sum[e]^T @ W1[e]  (stationary [128h,1], moving bf16 rows)
        pay = pool.tile([1, VPART], F32)
        DK = D // VK
        for e in range(E):
            for k in range(VK):
                vch = psum.tile([1, DK], F32, name="vch", tag="vch", bufs=2)
                nc.tensor.matmul(
                    vch[:],
                    w2s[:, e : e + 1],
                    w1_sb[:, e, k * DK : (k + 1) * DK],
                    start=True,
                    stop=False,
                )
                nc.tensor.matmul(
                    vch[:],
                    w2sl[:, e : e + 1],
                    w1_sb[:, e, k * DK : (k + 1) * DK],
                    start=False,
                    stop=True,
                )
                dst = pay[0:1, e * D + k * DK : e * D + (k + 1) * DK]
                if k % 2 == 0:
                    nc.vector.tensor_copy(dst, vch[:])
                else:
                    nc.scalar.copy(dst, vch[:])

        # c[e] = b1[e].w2sum[e] + sum(b2[e])   (b1/b2 are zeros per spec,
        # kept for generality; bf16 b1 path is accuracy-irrelevant here)
        b1ps = psum.tile([1, E], F32)
        for e in range(E):
            nc.tensor.matmul(
                b1ps[0:1, e : e + 1],
                w2s[:, e : e + 1],
                b1_sb[:, e : e + 1],
                start=True,
                stop=True,
            )
        b2s = pool.tile([1, E], F32)
        for e in range(E):
            nc.vector.reduce_sum(
                b2s[0:1, e : e + 1], b2_sb[0:1, e * DC : (e + 1) * DC], axis=AX.X
            )
        for e in range(E):
            nc.vector.tensor_add(
                pay[0:1, 2 * D + e : 2 * D + e + 1],
                b1ps[0:1, e : e + 1],
                b2s[0:1, e : e + 1],
            )
        nc.sync.dma_start(vout[:], pay[:])


def emit_phase_b(nc, tc, io):
    """hi/lo bf16 logits+s stream, gate/select, row log_softmax."""
    x2, m8d, csum_d, out = io["x2"], io["m8"], io["csum"], io["out"]
    with (
        tc.tile_pool(name="main", bufs=1) as pool,
        tc.tile_pool(name="psum", bufs=1, space="PSUM") as psum,
    ):
        # m8 first (first matmul needs it), then the x chunks in n-order
        # alternating the two HWDGE rings.
        m8 = pool.tile([P, NB, 8], BF16)
        nc.sync.dma_start(m8[:], m8d)
        x_sb = pool.tile([P, NB, 2, TB], BF16)
        qs = [nc.sync, nc.scalar]
        chunks = [
            (1, 0, 1), (0, 1, 2),
            (1, 2, 4), (0, 4, 6),
            (1, 6, 9), (0, 9, 12),
            (1, 12, 14), (0, 14, 16),
        ]
        for q, lo, hi in chunks:
            qs[q].dma_start(x_sb[:, lo:hi], x2[:, lo:hi])
        csum = pool.tile([1, E], F32)
        nc.gpsimd.dma_start(csum[:], csum_d)

        # preload ACT tables (Exp, Ln) off the critical path; keep ALL copy
        # work off the scalar engine so these tables are never evicted
        warm = pool.tile([1, 2], F32)
        nc.gpsimd.memset(warm[:], 1.0)
        wz = pool.tile([1, 2], F32)
        nc.scalar.activation(wz[:], warm[:], AF.Exp)
        nc.scalar.activation(wz[:], warm[:], AF.Ln)

        ident = pool.tile([P, P], F32)
        make_identity(nc, ident[:])
        # c broadcast tile on every partition
        cb2 = pool.tile([P, E], F32)
        nc.gpsimd.partition_broadcast(cb2[:], csum[0:1, :])

        # psum [8, TB] accumulates xh- and xl-streams against the M=8
        # stationary [wgh0 wgh1 wgl0 wgl1 vh0 vh1 vl0 vl1] per d-block
        ps8 = psum.tile([8, TB], F32)
        for n in range(NB):
            for hl in range(2):
                nc.tensor.matmul(
                    ps8[:],
                    m8[:, n, :],
                    x_sb[:, n, hl, :],
                    start=(n == 0 and hl == 0),
                    stop=(n == NB - 1 and hl == 1),
                )
        sbl = pool.tile([8, TB], F32)
        nc.vector.tensor_copy(sbl[:], ps8[:])

        moe_sb = pool.tile([P, NG], F32)
        for g in range(NG):
            tpl = psum.tile([P, 8], F32, name=f"tpl_{g}", tag="tp", bufs=2)
            nc.tensor.transpose(tpl[:], sbl[0:8, g * P : (g + 1) * P], ident[0:8, 0:8])
            t8 = pool.tile([P, 8], F32, name=f"t8_{g}")
            nc.vector.tensor_copy(t8[:], tpl[:])
            t4 = pool.tile([P, 4], F32, name=f"t4_{g}")
            nc.vector.tensor_add(t4[:, 0:2], t8[:, 0:2], t8[:, 2:4])  # logits
            nc.vector.tensor_add(t4[:, 2:4], t8[:, 4:6], t8[:, 6:8])  # s
            nc.vector.tensor_add(t4[:, 2:4], t4[:, 2:4], cb2[:])
            negm = pool.tile([P, 1], F32, name=f"negm_{g}")
            nc.vector.reduce_max(negm[:], t4[:, 0:2], axis=AX.X, negate=True)
            z = pool.tile([P, E], F32, name=f"z_{g}")
            den = pool.tile([P, 1], F32, name=f"den_{g}")
            nc.scalar.activation(z[:], t4[:, 0:2], AF.Exp, bias=negm[:], accum_out=den[:])
            rec = pool.tile([P, 1], F32, name=f"rec_{g}")
            nc.vector.reciprocal(rec[:], den[:])
            zmax = pool.tile([P, 1], F32, name=f"zmax_{g}")
            nc.vector.reduce_max(zmax[:], z[:], axis=AX.X)
            gate = pool.tile([P, 1], F32, name=f"gate_{g}")
            nc.vector.tensor_mul(gate[:], zmax[:], rec[:])
            mask = pool.tile([P, 1], F32, name=f"mask_{g}")
            nc.vector.tensor_tensor(mask[:], t4[:, 0:1], t4[:, 1:2], op=ALU.is_ge)
            sdiff = pool.tile([P, 1], F32, name=f"sdiff_{g}")
            nc.vector.tensor_sub(sdiff[:], t4[:, 2:3], t4[:, 3:4])
            ssel = pool.tile([P, 1], F32, name=f"ssel_{g}")
            nc.vector.tensor_mul(ssel[:], mask[:], sdiff[:])
            nc.vector.tensor_add(ssel[:], ssel[:], t4[:, 3:4])
            nc.vector.tensor_mul(moe_sb[:, g : g + 1], gate[:], ssel[:])

        # row log_softmax over all 512 tokens, via PE transposes
        tp4 = psum.tile([NG, P], F32)
        nc.tensor.transpose(tp4[:], moe_sb[:], ident[:])
        sb4t = pool.tile([NG, P], F32)
        nc.vector.tensor_copy(sb4t[:], tp4[:])
        m4p = pool.tile([NG, 1], F32)
        nc.vector.reduce_max(m4p[:], sb4t[:], axis=AX.X)
        m1p = psum.tile([1, NG], F32, name="m1p", tag="t1", bufs=2)
        nc.tensor.transpose(m1p[:], m4p[:], ident[0:NG, 0:NG])
        negm2 = pool.tile([1, 1], F32)
        nc.vector.reduce_max(negm2[:], m1p[:], axis=AX.X, negate=True)
        negm4 = pool.tile([NG, 1], F32)
        nc.gpsimd.partition_broadcast(negm4[:], negm2[:])
        e4 = pool.tile([NG, P], F32)
        s4 = pool.tile([NG, 1], F32)
        nc.scalar.activation(e4[:], sb4t[:], AF.Exp, bias=negm4[:], accum_out=s4[:])
        # reload the Ln table NOW (the Exp uses above evicted it) so the real
        # Ln below table-hits; overlaps the transpose+reduce on other engines
        nc.scalar.activation(wz[:], warm[:], AF.Ln)
        s1p = psum.tile([1, NG], F32, name="s1p", tag="t1", bufs=2)
        nc.tensor.transpose(s1p[:], s4[:], ident[0:NG, 0:NG])
        ssum = pool.tile([1, 1], F32)
        nc.vector.reduce_sum(ssum[:], s1p[:], axis=AX.X)
        logs = pool.tile([1, 1], F32)
        nc.scalar.activation(logs[:], ssum[:], AF.Ln)
        shift = pool.tile([1, 1], F32)
        nc.vector.tensor_sub(shift[:], negm2[:], logs[:])
        shift4 = pool.tile([NG, 1], F32)
        nc.gpsimd.partition_broadcast(shift4[:], shift[:])
        res4 = pool.tile([NG, P], F32)
        nc.vector.tensor_scalar_add(res4[:], sb4t[:], shift4[:])
        nc.sync.dma_start(out.rearrange("x (g p) -> g (x p)", p=P), res4[:])


_CACHED = {}


def build_program(which):
    if which in _CACHED:
        return _CACHED[which]
    nc = bacc.Bacc(
        "TRN2",
        target_bir_lowering=False,
        debug=False,
        enable_asserts=False,
        num_devices=NCORES,
    )
    if which == "a":
        io = {
            "w2d": nc.dram_tensor("w2d", [P, NB, E, HC], BF16, kind="ExternalInput").ap(),
            "w1t": nc.dram_tensor("w1t", [P, E, D], BF16, kind="ExternalInput").ap(),
            "b1t": nc.dram_tensor("b1t", [P, E], BF16, kind="ExternalInput").ap(),
            "b2c": nc.dram_tensor("b2c", [1, E * DC], F32, kind="ExternalInput").ap(),
            "vout": nc.dram_tensor("vout", [1, VPART], F32, kind="ExternalOutput").ap(),
        }
        emit = emit_phase_a
    else:
        io = {
            "x2": nc.dram_tensor("x2", [P, NB, 2, TB], BF16, kind="ExternalInput").ap(),
            "m8": nc.dram_tensor("m8", [P, NB, 8], BF16, kind="ExternalInput").ap(),
            "csum": nc.dram_tensor("csum", [1, E], F32, kind="ExternalInput").ap(),
            "out": nc.dram_tensor("out", [1, TB], F32, kind="ExternalOutput").ap(),
        }
        emit = emit_phase_b
    with tile.TileContext(nc) as tc:
        emit(nc, tc, io)
    nc.compile()
    _CACHED[which] = nc
    return nc


def _hi_lo(a):
    import ml_dtypes

    hi = a.astype(ml_dtypes.bfloat16)
    lo = (a - hi.astype(np.float32)).astype(ml_dtypes.bfloat16)
    return hi, lo


def shard_inputs_a(Wg, W1, b1, W2, b2):
    import ml_dtypes

    bf16 = ml_dtypes.bfloat16
    W1 = np.asarray(W1, np.float32)
    b1 = np.asarray(b1, np.float32)
    W2 = np.asarray(W2, np.float32)
    b2 = np.asarray(b2, np.float32)
    in_maps = []
    for c in range(NCORES):
        hs, he = c * HC, (c + 1) * HC
        # w2d[p, n, e, h] = W2[e, hs+h, p*16+n]  (d = p*16 + n: 8KB runs)
        w2d = np.ascontiguousarray(
            W2[:, hs:he, :].transpose(2, 0, 1).reshape(P, NB, E, HC).astype(bf16)
        )
        # w1t[h, e, d] = W1[e, d, hs+h]
        w1t = np.ascontiguousarray(W1[:, :, hs:he].transpose(2, 0, 1).astype(bf16))
        b1t = np.ascontiguousarray(b1[:, hs:he].T.astype(bf16))
        in_maps.append(
            {
                "w2d": w2d,
                "w1t": w1t,
                "b1t": b1t,
                "b2c": np.ascontiguousarray(
                    b2[:, c * DC : (c + 1) * DC].reshape(1, E * DC)
                ),
            }
        )
    return in_maps


def shard_inputs_b(x, Wg, vpart_sum):
    x = np.asarray(x, np.float32).reshape(B * T, D)
    Wg = np.asarray(Wg, np.float32)
    v = np.asarray(vpart_sum, np.float32).reshape(-1)[: 2 * D].reshape(E, D)
    csum = np.ascontiguousarray(
        np.asarray(vpart_sum, np.float32).reshape(-1)[2 * D : 2 * D + E].reshape(1, E)
    )
    # m8[p, n, :] = [wgh0 wgh1 wgl0 wgl1 vh0 vh1 vl0 vl1] at d = n*128 + p
    wgh, wgl = _hi_lo(Wg)  # [D, E]
    vh, vl = _hi_lo(v.T)  # [D, E]
    m8 = np.concatenate([wgh, wgl, vh, vl], axis=1)  # [D, 8]
    m8 = np.ascontiguousarray(m8.reshape(NB, P, 8).transpose(1, 0, 2))
    in_maps = []
    for c in range(NCORES):
        row = c % B
        xr = x[row * TB : (row + 1) * TB, :]  # [TB, D]
        xh, xl = _hi_lo(xr.T)  # [D, TB]
        # x2[p, n, hl, t] at d = n*128 + p
        x2 = np.ascontiguousarray(
            np.stack([xh, xl], axis=1).reshape(NB, P, 2, TB).transpose(1, 0, 2, 3)
        )
        in_maps.append({"x2": x2, "m8": m8, "csum": csum})
    return in_maps


def run_a(in_maps, **kwargs):
    return bass_utils.run_bass_kernel_spmd(
        build_program("a"), in_maps, core_ids=list(range(NCORES)), **kwargs
    )


def run_b(in_maps, **kwargs):
    return bass_utils.run_bass_kernel_spmd(
        build_program("b"), in_maps, core_ids=list(range(NCORES)), **kwargs
    )


def kernel(x, Wg, W1, b1, W2, b2):
    res_a = run_a(shard_inputs_a(Wg, W1, b1, W2, b2))
    # cross-core combine: sum of the 8 per-core partials (the gather/reshard
    # step between the two launches; 16KB, no model math beyond the reduction)
    vpart = np.sum([res_a.results[c]["vout"] for c in range(NCORES)], axis=0)
    vpart = np.ascontiguousarray(vpart, np.float32)
    res_b = run_b(shard_inputs_b(x, Wg, vpart))
    return np.concatenate([res_b.results[b]["out"] for b in range(B)], axis=0)


# revision 18
# speedup vs baseline: 1.2116x; 1.0778x over previous
"""Trainium2 Bass kernel for nn_ExampleModel_1116691497724 (moe_routing).

Math: the reference returns log_softmax_T( sum_D(moe_out) ), and sum_D
collapses the expert FFN to a dot product:
    sum_d (h @ W2[e] + b2[e]) = h . w2sum[e] + sum(b2[e]),  w2sum[e] = W2[e] @ 1
    (x @ W1[e] + b1[e]) . w2sum[e] = x . v[e] + c[e]
with v[e] = W1[e] @ w2sum[e]  (a [D] vector) and scalar
c[e] = b1[e].w2sum[e] + sum(b2[e]).  Then per token:
    s_e = x . v[e] + c[e],  logits = x @ Wg
    moe_sum = max(softmax(logits)) * s_argmax(logits)
    out = log_softmax over tokens (per batch row) of moe_sum.

Distribution over 8 cores, two launches (an on-device ncfw collective costs
~65us of barrier/trigger latency on this runtime, far more than a second
launch; the 16KB cross-core combine of v-partials happens on the host between
launches — the host does only that partial sum, all real math stays on device):
  launch A (expert-parallel over H): core c owns h-chunk [128c,128c+128) of
    both experts.  W2 ships bf16 d-major so w2sum is a PE ones-matmul
    (stationary [128d,128h] tiles, FWL bf16 loads), W1 ships bf16 h-major so
    v = w2sum^T-stationary @ W1-moving streams at 1 cyc/row.  Outputs
    [v0 | v1 | c0 c1] partials (16KB); host sums the 8 payloads.
  launch B (token-parallel): core c owns batch row c%4 (512 tokens).  x ships
    as a bf16 hi/lo pair (x = xh + xl exactly to ~2^-17), and one M=8
    stationary [wgh0 wgh1 wgl0 wgl1 vh0 vh1 vl0 vl1] (bf16 hi/lo of Wg and v)
    is streamed by xh then xl at 1 cyc/row: all four cross products accumulate
    in fp32 PSUM, so logits are fp32-grade (argmax must match the reference;
    bf16-only logits would flip near-boundary tokens) while the whole PE
    stream is 4x cheaper than an fp32 x stream.  l_e = col_e+col_{2+e},
    s_e = col_{4+e}+col_{6+e}+c_e after a PE transpose to token-major; then
    gate/select per token and the row log_softmax via PE transposes exactly
    as before (no cross-partition DMA).  Host takes rows from cores 0..3.

Scheduling: both launches issue the big HBM loads on the two HWDGE rings
(SP via nc.sync, ACT via nc.scalar) as their first instructions, before any
ACT-table load can head-of-line block a ring.  All hi/lo splits, transposes
and packing happen on the host (input reformatting only).
"""

import sys

import numpy as np

for _p in ("/opt/trn_rl_repo",):
    if _p not in sys.path:
        sys.path.append(_p)

import concourse.bass as bass  # noqa: E402
import concourse.mybir as mybir  # noqa: E402
import concourse.tile as tile  # noqa: E402
from concourse import bacc, bass_utils  # noqa: E402
from concourse.masks import make_identity  # noqa: E402

# Problem shape (hardcoded per spec).
B, T, D, H, E = 4, 512, 2048, 1024, 2
P = 128
NCORES = 8
TB = T  # tokens per core = one batch row
NB = D // P  # 16 d-blocks
HC = H // NCORES  # 128 h-chunk per expert per core
NG = TB // P  # 4 token groups per core
DC = D // NCORES  # 256 b2 columns per core
VK = 4  # v computed in VK chunks of D/VK columns
F32 = mybir.dt.float32
BF16 = mybir.dt.bfloat16
AX = mybir.AxisListType
AF = mybir.ActivationFunctionType
ALU = mybir.AluOpType

# launch A output: [128, E*NB + E] f32 — v partition-major (col e*NB+n on
# partition p holds v[e, n*128+p]) plus c0,c1 on partition 0
VCOLS = E * NB + E


def emit_phase_a(nc, tc, io):
    """w2sum (PE ones-matmul) + partial v for this core's H-chunk."""
    w2d, w1t, b1t, b2c, vout = io["w2d"], io["w1t"], io["b1t"], io["b2c"], io["vout"]
    with (
        tc.tile_pool(name="main", bufs=1) as pool,
        tc.tile_pool(name="psum", bufs=1, space="PSUM") as psum,
    ):
        # Big loads first on both HWDGE rings.  W2 (d-major) gates the
        # reduce, so it takes a whole ring; W1 per-expert on the other so
        # the e=0 v-chain can start as soon as its half lands.
        w2_sb = pool.tile([P, NB, E, HC], BF16)
        nc.sync.dma_start(w2_sb[:], w2d[:])
        w1_sb = pool.tile([P, E, D], BF16)
        nc.scalar.dma_start(w1_sb[:, 0, :], w1t[:, 0, :])
        nc.scalar.dma_start(w1_sb[:, 1, :], w1t[:, 1, :])
        b1_sb = pool.tile([P, E], BF16)
        nc.gpsimd.dma_start(b1_sb[:], b1t)
        b2_sb = pool.tile([1, E * DC], F32)
        nc.gpsimd.dma_start(b2_sb[:], b2c)

        ones = pool.tile([P, 1], BF16)
        nc.gpsimd.memset(ones[:], 1.0)

        # PE warm-up during the DMA window: sustained dummy matmuls ramp the
        # HAM clock 1.2 -> 2.4 GHz so the real v-chain streams 2x faster
        dum = pool.tile([P, 512], BF16)
        nc.gpsimd.memset(dum[:], 0.25)
        wps = psum.tile([1, 512], F32, name="warmps")
        for _ in range(8):
            nc.tensor.matmul(wps[:], ones[:], dum[:], start=True, stop=True)

        # w2sum[e, h] via PE: stationary [128d, 128h] tiles, moving ones.
        w2ps = [psum.tile([P, 1], F32, name=f"w2ps_{e}") for e in range(E)]
        for n in range(NB):
            for e in range(E):
                nc.tensor.matmul(
                    w2ps[e][:],
                    w2_sb[:, n, e, :],
                    ones[:],
                    start=(n == 0),
                    stop=(n == NB - 1),
                )
        # w2sum as a bf16 hi+lo column pair per expert (a single-bf16 cast
        # would dominate the accuracy budget)
        w2sf = pool.tile([P, E], F32)
        w2hl = pool.tile([P, E, 2], BF16)
        w2r32 = pool.tile([P, E], F32)
        for e in range(E):
            nc.vector.tensor_copy(w2sf[:, e : e + 1], w2ps[e][:])
            nc.vector.tensor_copy(w2hl[:, e, 0:1], w2ps[e][:])
        nc.vector.tensor_copy(w2r32[:], w2hl[:, :, 0])
        w2lo = pool.tile([P, E], F32)
        nc.vector.tensor_sub(w2lo[:], w2sf[:], w2r32[:])
        nc.vector.tensor_copy(w2hl[:, :, 1], w2lo[:])

        # v[e] = W1[e]^T-stationary @ [w2sum_hi | w2sum_lo]-moving: v comes
        # out PARTITION-major ([128, NB, 2] per expert), so the PSUM->SBUF
        # hop is two wide copies, not eight single-partition crawls
        pay3 = pool.tile([P, E, NB], F32)
        for e in range(E):
            vps = psum.tile([P, NB, 2], F32, name=f"vps_{e}")
            for n in range(NB):
                nc.tensor.matmul(
                    vps[:, n, :],
                    w1_sb[:, e, n * P : (n + 1) * P],
                    w2hl[:, e, :],
                    start=True,
                    stop=True,
                )
            vt = pool.tile([P, NB, 2], F32, name=f"vt_{e}")
            nc.vector.tensor_copy(vt[:], vps[:])
            nc.vector.tensor_add(
                pay3[:, e, :, None], vt[:, :, 0:1], vt[:, :, 1:2]
            )
        nc.sync.dma_start(vout[:, 0 : E * NB], pay3[:])

        # c[e] = b1[e].w2sum[e] + sum(b2[e])   (b1/b2 are zeros per spec,
        # kept for generality; bf16 b1 path is accuracy-irrelevant here)
        b1ps = psum.tile([1, E], F32)
        for e in range(E):
            nc.tensor.matmul(
                b1ps[0:1, e : e + 1],
                w2hl[:, e, 0:1],
                b1_sb[:, e : e + 1],
                start=True,
                stop=True,
            )
        b2s = pool.tile([1, E], F32)
        for e in range(E):
            nc.vector.reduce_sum(
                b2s[0:1, e : e + 1], b2_sb[0:1, e * DC : (e + 1) * DC], axis=AX.X
            )
        cpay = pool.tile([1, E], F32)
        nc.vector.tensor_add(cpay[:], b1ps[:], b2s[:])
        nc.gpsimd.dma_start(vout[0:1, E * NB : E * NB + E], cpay[:])


def emit_phase_b(nc, tc, io):
    """hi/lo bf16 logits+s stream, gate/select, row log_softmax."""
    x2, m8d, csum_d, out = io["x2"], io["m8"], io["csum"], io["out"]
    with (
        tc.tile_pool(name="main", bufs=1) as pool,
        tc.tile_pool(name="psum", bufs=1, space="PSUM") as psum,
    ):
        # m8 first (first matmul needs it), then the x chunks in n-order
        # alternating the two HWDGE rings.
        m8 = pool.tile([P, NB, 8], BF16)
        nc.sync.dma_start(m8[:], m8d)
        x_sb = pool.tile([P, NB, 2, TB], BF16)
        qs = [nc.sync, nc.scalar]
        chunks = [
            (1, 0, 1), (0, 1, 2),
            (1, 2, 4), (0, 4, 6),
            (1, 6, 9), (0, 9, 12),
            (1, 12, 14), (0, 14, 16),
        ]
        for q, lo, hi in chunks:
            qs[q].dma_start(x_sb[:, lo:hi], x2[:, lo:hi])
        csum = pool.tile([1, E], F32)
        nc.gpsimd.dma_start(csum[:], csum_d)

        # PE warm-up during the DMA window (HAM ramp, see phase A)
        dum = pool.tile([P, 512], BF16)
        nc.gpsimd.memset(dum[:], 0.25)
        st1 = pool.tile([P, 1], BF16)
        nc.gpsimd.memset(st1[:], 0.5)
        wps = psum.tile([1, 512], F32, name="warmps")
        for _ in range(8):
            nc.tensor.matmul(wps[:], st1[:], dum[:], start=True, stop=True)

        # preload the Sigmoid table (gate).  The table cache holds ~one
        # entry, so Exp/Ln are each prefetched just after the preceding
        # table's last use, hidden under DVE/PE work.  Reading csum (not a
        # const) delays this load until after the ring triggers, so it
        # can't head-of-line block the x DMA.
        wz = pool.tile([1, E], F32)
        nc.scalar.activation(wz[:], csum[0:1, :], AF.Sigmoid)

        ident = pool.tile([P, P], F32)
        make_identity(nc, ident[:])
        # c broadcast tile on every partition, replicated per token group
        cb8 = pool.tile([P, NG, E], F32)
        for g in range(NG):
            nc.gpsimd.partition_broadcast(cb8[:, g, :], csum[0:1, :])

        # psum [8, TB] accumulates xh- and xl-streams against the M=8
        # stationary [wgh0 wgh1 wgl0 wgl1 vh0 vh1 vl0 vl1] per d-block
        ps8 = psum.tile([8, TB], F32)
        for n in range(NB):
            for hl in range(2):
                nc.tensor.matmul(
                    ps8[:],
                    m8[:, n, :],
                    x_sb[:, n, hl, :],
                    start=(n == 0 and hl == 0),
                    stop=(n == NB - 1 and hl == 1),
                )
        sbl = pool.tile([8, TB], F32)
        nc.vector.tensor_copy(sbl[:], ps8[:])

        # token-major via 4 PE transposes into one PSUM tile, then ALL
        # gating math batched across the 4 groups in single strided DVE ops.
        # gate = softmax(l).max == sigmoid(|l0-l1|), mask = (l0 >= l1).
        tpa = psum.tile([P, NG, 8], F32)
        for g in range(NG):
            nc.tensor.transpose(
                tpa[:, g, :], sbl[0:8, g * P : (g + 1) * P], ident[0:8, 0:8]
            )
        t8a = pool.tile([P, NG, 8], F32)
        nc.vector.tensor_copy(t8a[:], tpa[:])
        l4 = pool.tile([P, NG, E], F32)
        nc.vector.tensor_add(l4[:], t8a[:, :, 0:2], t8a[:, :, 2:4])  # logits
        s4p = pool.tile([P, NG, E], F32)
        nc.vector.tensor_add(s4p[:], t8a[:, :, 4:6], t8a[:, :, 6:8])  # s
        nc.vector.tensor_add(s4p[:], s4p[:], cb8[:])
        dl = pool.tile([P, NG, 1], F32)
        nc.vector.tensor_sub(dl[:], l4[:, :, 0:1], l4[:, :, 1:2])
        ndl = pool.tile([P, NG, 1], F32)
        nc.vector.tensor_scalar_mul(ndl[:], dl[:], -1.0)
        absdl = pool.tile([P, NG, 1], F32)
        nc.vector.tensor_tensor(absdl[:], dl[:], ndl[:], op=ALU.max)
        gate = pool.tile([P, NG, 1], F32)
        nc.scalar.activation(gate[:], absdl[:], AF.Sigmoid)
        # prefetch the Exp table (for the row softmax) while DVE finishes
        # the select math
        nc.scalar.activation(wz[:], csum[0:1, :], AF.Exp)
        mask = pool.tile([P, NG, 1], F32)
        nc.vector.tensor_scalar(mask[:], dl[:], 0.0, None, op0=ALU.is_ge)
        sdiff = pool.tile([P, NG, 1], F32)
        nc.vector.tensor_sub(sdiff[:], s4p[:, :, 0:1], s4p[:, :, 1:2])
        ssel = pool.tile([P, NG, 1], F32)
        nc.vector.tensor_mul(ssel[:], mask[:], sdiff[:])
        nc.vector.tensor_add(ssel[:], ssel[:], s4p[:, :, 1:2])
        moe_sb = pool.tile([P, NG], F32)
        nc.vector.tensor_mul(moe_sb[:, :, None], gate[:], ssel[:])

        # row log_softmax over all 512 tokens, via PE transposes
        tp4 = psum.tile([NG, P], F32)
        nc.tensor.transpose(tp4[:], moe_sb[:], ident[:])
        sb4t = pool.tile([NG, P], F32)
        nc.vector.tensor_copy(sb4t[:], tp4[:])
        m4p = pool.tile([NG, 1], F32)
        nc.vector.reduce_max(m4p[:], sb4t[:], axis=AX.X)
        m1p = psum.tile([1, NG], F32, name="m1p", tag="t1", bufs=2)
        nc.tensor.transpose(m1p[:], m4p[:], ident[0:NG, 0:NG])
        negm2 = pool.tile([1, 1], F32)
        nc.vector.reduce_max(negm2[:], m1p[:], axis=AX.X, negate=True)
        negm4 = pool.tile([NG, 1], F32)
        nc.gpsimd.partition_broadcast(negm4[:], negm2[:])
        e4 = pool.tile([NG, P], F32)
        s4 = pool.tile([NG, 1], F32)
        nc.scalar.activation(e4[:], sb4t[:], AF.Exp, bias=negm4[:], accum_out=s4[:])
        # load the Ln table NOW so the real Ln below table-hits; overlaps
        # the transpose+reduce running on other engines
        nc.scalar.activation(wz[:], csum[0:1, :], AF.Ln)
        s1p = psum.tile([1, NG], F32, name="s1p", tag="t1", bufs=2)
        nc.tensor.transpose(s1p[:], s4[:], ident[0:NG, 0:NG])
        ssum = pool.tile([1, 1], F32)
        nc.vector.reduce_sum(ssum[:], s1p[:], axis=AX.X)
        logs = pool.tile([1, 1], F32)
        nc.scalar.activation(logs[:], ssum[:], AF.Ln)
        shift = pool.tile([1, 1], F32)
        nc.vector.tensor_sub(shift[:], negm2[:], logs[:])
        shift4 = pool.tile([NG, 1], F32)
        nc.gpsimd.partition_broadcast(shift4[:], shift[:])
        res4 = pool.tile([NG, P], F32)
        nc.vector.tensor_scalar_add(res4[:], sb4t[:], shift4[:])
        nc.sync.dma_start(out.rearrange("x (g p) -> g (x p)", p=P), res4[:])


_CACHED = {}


def build_program(which):
    if which in _CACHED:
        return _CACHED[which]
    nc = bacc.Bacc(
        "TRN2",
        target_bir_lowering=False,
        debug=False,
        enable_asserts=False,
        num_devices=NCORES,
    )
    if which == "a":
        io = {
            "w2d": nc.dram_tensor("w2d", [P, NB, E, HC], BF16, kind="ExternalInput").ap(),
            "w1t": nc.dram_tensor("w1t", [P, E, D], BF16, kind="ExternalInput").ap(),
            "b1t": nc.dram_tensor("b1t", [P, E], BF16, kind="ExternalInput").ap(),
            "b2c": nc.dram_tensor("b2c", [1, E * DC], F32, kind="ExternalInput").ap(),
            "vout": nc.dram_tensor("vout", [P, VCOLS], F32, kind="ExternalOutput").ap(),
        }
        emit = emit_phase_a
    else:
        io = {
            "x2": nc.dram_tensor("x2", [P, NB, 2, TB], BF16, kind="ExternalInput").ap(),
            "m8": nc.dram_tensor("m8", [P, NB, 8], BF16, kind="ExternalInput").ap(),
            "csum": nc.dram_tensor("csum", [1, E], F32, kind="ExternalInput").ap(),
            "out": nc.dram_tensor("out", [1, TB], F32, kind="ExternalOutput").ap(),
        }
        emit = emit_phase_b
    with tile.TileContext(nc) as tc:
        emit(nc, tc, io)
    nc.compile()
    _CACHED[which] = nc
    return nc


def _hi_lo(a):
    import ml_dtypes

    hi = a.astype(ml_dtypes.bfloat16)
    lo = (a - hi.astype(np.float32)).astype(ml_dtypes.bfloat16)
    return hi, lo


def shard_inputs_a(Wg, W1, b1, W2, b2):
    import ml_dtypes

    bf16 = ml_dtypes.bfloat16
    W1 = np.asarray(W1, np.float32)
    b1 = np.asarray(b1, np.float32)
    W2 = np.asarray(W2, np.float32)
    b2 = np.asarray(b2, np.float32)
    in_maps = []
    for c in range(NCORES):
        hs, he = c * HC, (c + 1) * HC
        # w2d[p, n, e, h] = W2[e, hs+h, p*16+n]  (d = p*16 + n: 8KB runs)
        w2d = np.ascontiguousarray(
            W2[:, hs:he, :].transpose(2, 0, 1).reshape(P, NB, E, HC).astype(bf16)
        )
        # w1t[h, e, d] = W1[e, d, hs+h]
        w1t = np.ascontiguousarray(W1[:, :, hs:he].transpose(2, 0, 1).astype(bf16))
        b1t = np.ascontiguousarray(b1[:, hs:he].T.astype(bf16))
        in_maps.append(
            {
                "w2d": w2d,
                "w1t": w1t,
                "b1t": b1t,
                "b2c": np.ascontiguousarray(
                    b2[:, c * DC : (c + 1) * DC].reshape(1, E * DC)
                ),
            }
        )
    return in_maps


def shard_inputs_b(x, Wg, vpart_sum):
    x = np.asarray(x, np.float32).reshape(B * T, D)
    Wg = np.asarray(Wg, np.float32)
    arr = np.asarray(vpart_sum, np.float32).reshape(P, VCOLS)
    vm = arr[:, : E * NB].reshape(P, E, NB)
    # v[e, n*128+p] = vm[p, e, n]
    v = np.stack([vm[:, e, :].T.reshape(-1) for e in range(E)])  # [E, D]
    csum = np.ascontiguousarray(arr[0:1, E * NB : E * NB + E])
    # m8[p, n, :] = [wgh0 wgh1 wgl0 wgl1 vh0 vh1 vl0 vl1] at d = n*128 + p
    wgh, wgl = _hi_lo(Wg)  # [D, E]
    vh, vl = _hi_lo(v.T)  # [D, E]
    m8 = np.concatenate([wgh, wgl, vh, vl], axis=1)  # [D, 8]
    m8 = np.ascontiguousarray(m8.reshape(NB, P, 8).transpose(1, 0, 2))
    in_maps = []
    for c in range(NCORES):
        row = c % B
        xr = x[row * TB : (row + 1) * TB, :]  # [TB, D]
        xh, xl = _hi_lo(xr.T)  # [D, TB]
        # x2[p, n, hl, t] at d = n*128 + p
        x2 = np.ascontiguousarray(
            np.stack([xh, xl], axis=1).reshape(NB, P, 2, TB).transpose(1, 0, 2, 3)
        )
        in_maps.append({"x2": x2, "m8": m8, "csum": csum})
    return in_maps


def run_a(in_maps, **kwargs):
    return bass_utils.run_bass_kernel_spmd(
        build_program("a"), in_maps, core_ids=list(range(NCORES)), **kwargs
    )


def run_b(in_maps, **kwargs):
    return bass_utils.run_bass_kernel_spmd(
        build_program("b"), in_maps, core_ids=list(range(NCORES)), **kwargs
    )


def kernel(x, Wg, W1, b1, W2, b2):
    res_a = run_a(shard_inputs_a(Wg, W1, b1, W2, b2))
    # cross-core combine: sum of the 8 per-core partials (the gather/reshard
    # step between the two launches; 16KB, no model math beyond the reduction)
    vpart = np.sum([res_a.results[c]["vout"] for c in range(NCORES)], axis=0)
    vpart = np.ascontiguousarray(vpart, np.float32)
    res_b = run_b(shard_inputs_b(x, Wg, vpart))
    return np.concatenate([res_b.results[b]["out"] for b in range(B)], axis=0)


# revision 28
# speedup vs baseline: 1.3534x; 1.1170x over previous
"""Trainium2 Bass kernel for nn_ExampleModel_1116691497724 (moe_routing).

Math: the reference returns log_softmax_T( sum_D(moe_out) ), and sum_D
collapses the expert FFN to a dot product:
    sum_d (h @ W2[e] + b2[e]) = h . w2sum[e] + sum(b2[e]),  w2sum[e] = W2[e] @ 1
    (x @ W1[e] + b1[e]) . w2sum[e] = x . v[e] + c[e]
with v[e] = W1[e] @ w2sum[e]  (a [D] vector) and scalar
c[e] = b1[e].w2sum[e] + sum(b2[e]).  Then per token:
    s_e = x . v[e] + c[e],  logits = x @ Wg
    moe_sum = max(softmax(logits)) * s_argmax(logits)
    out = log_softmax over tokens (per batch row) of moe_sum.

Distribution over 8 cores, two launches (an on-device ncfw collective costs
~65us of barrier/trigger latency on this runtime, far more than a second
launch; the 16KB cross-core combine of v-partials happens on the host between
launches — the host does only that partial sum, all real math stays on device):
  launch A (expert-parallel over H): core c owns h-chunk [128c,128c+128) of
    both experts.  W2 ships bf16 d-major so w2sum is a PE ones-matmul
    (stationary [128d,128h] tiles, FWL bf16 loads), W1 ships bf16 h-major so
    v = w2sum^T-stationary @ W1-moving streams at 1 cyc/row.  Outputs
    [v0 | v1 | c0 c1] partials (16KB); host sums the 8 payloads.
  launch B (token-parallel): core c owns batch row c%4 (512 tokens).  x ships
    as a bf16 hi/lo pair (x = xh + xl exactly to ~2^-17), and one M=8
    stationary [wgh0 wgh1 wgl0 wgl1 vh0 vh1 vl0 vl1] (bf16 hi/lo of Wg and v)
    is streamed by xh then xl at 1 cyc/row: all four cross products accumulate
    in fp32 PSUM, so logits are fp32-grade (argmax must match the reference;
    bf16-only logits would flip near-boundary tokens) while the whole PE
    stream is 4x cheaper than an fp32 x stream.  l_e = col_e+col_{2+e},
    s_e = col_{4+e}+col_{6+e}+c_e after a PE transpose to token-major; then
    gate/select per token and the row log_softmax via PE transposes exactly
    as before (no cross-partition DMA).  Host takes rows from cores 0..3.

Scheduling: both launches issue the big HBM loads on the two HWDGE rings
(SP via nc.sync, ACT via nc.scalar) as their first instructions, before any
ACT-table load can head-of-line block a ring.  All hi/lo splits, transposes
and packing happen on the host (input reformatting only).
"""

import sys

import numpy as np

for _p in ("/opt/trn_rl_repo",):
    if _p not in sys.path:
        sys.path.append(_p)

import concourse.bass as bass  # noqa: E402
import concourse.mybir as mybir  # noqa: E402
import concourse.tile as tile  # noqa: E402
from concourse import bacc, bass_utils  # noqa: E402
from concourse.masks import make_identity  # noqa: E402

# Problem shape (hardcoded per spec).
B, T, D, H, E = 4, 512, 2048, 1024, 2
P = 128
NCORES = 8
TB = T  # tokens per core = one batch row
NB = D // P  # 16 d-blocks
HC = H // NCORES  # 128 h-chunk per expert per core
NG = TB // P  # 4 token groups per core
DC = D // NCORES  # 256 b2 columns per core
VK = 4  # v computed in VK chunks of D/VK columns
NSPLIT = 12  # xl d-blocks 0..NSPLIT-1 stream in launch A, the rest in B
XB = NB + (NB - NSPLIT)  # moving blocks in launch B: xh 0..15 then xl NSPLIT..15
F32 = mybir.dt.float32
BF16 = mybir.dt.bfloat16
AX = mybir.AxisListType
AF = mybir.ActivationFunctionType
ALU = mybir.AluOpType

# launch A output: [128, E*NB + E] f32 — v partition-major (col e*NB+n on
# partition p holds v[e, n*128+p]) plus c0,c1 on partition 0
VCOLS = E * NB + E


def emit_phase_a(nc, tc, io):
    """w2sum (PE ones-matmul) + partial v for this core's H-chunk."""
    w2d, w1t, b1t, b2c = io["w2d"], io["w1t"], io["b1t"], io["b2c"]
    xlr, m4a, vout, lo_out = io["xlr"], io["m4a"], io["vout"], io["lo_out"]
    with (
        tc.tile_pool(name="main", bufs=1) as pool,
        tc.tile_pool(name="psum", bufs=1, space="PSUM") as psum,
    ):
        # Big loads first on both HWDGE rings, balanced ~1.75MB each.  W2
        # (d-major) gates the reduce so it leads ring 0; W1 per-expert leads
        # ring 1 so the v-chain starts early; the xl halves trail both rings.
        HS = NSPLIT // 2
        w2_sb = pool.tile([P, NB, E, HC], BF16)
        nc.sync.dma_start(w2_sb[:], w2d[:])
        w1_sb = pool.tile([P, E, D], BF16)
        nc.scalar.dma_start(w1_sb[:, 0, :], w1t[:, 0, :])
        nc.scalar.dma_start(w1_sb[:, 1, :], w1t[:, 1, :])
        xl_sb = pool.tile([P, NSPLIT, TB], BF16)
        nc.sync.dma_start(xl_sb[:, 0:HS], xlr[:, 0:HS])
        nc.scalar.dma_start(xl_sb[:, HS:NSPLIT], xlr[:, HS:NSPLIT])
        b1_sb = pool.tile([P, E], BF16)
        nc.gpsimd.dma_start(b1_sb[:], b1t)
        b2_sb = pool.tile([1, E * DC], F32)
        nc.gpsimd.dma_start(b2_sb[:], b2c)
        m4_sb = pool.tile([P, NSPLIT, 4], BF16)
        nc.gpsimd.dma_start(m4_sb[:], m4a[:])

        ones = pool.tile([P, 1], BF16)
        nc.vector.memset(ones[:], 1.0)

        # PE warm-up during the DMA window: sustained dummy matmuls ramp the
        # HAM clock so the real streams run fast (memsets on DVE, whose
        # queue frees up earliest)
        dum = pool.tile([P, 512], BF16)
        nc.vector.memset(dum[:], 0.25)
        wps = psum.tile([1, 512], F32, name="warmps")
        for _ in range(6):
            nc.tensor.matmul(wps[:], ones[:], dum[:], start=True, stop=True)

        # w2sum[e, h] via PE: stationary [128d, 128h] tiles, moving ones.
        w2ps = [psum.tile([P, 1], F32, name=f"w2ps_{e}") for e in range(E)]
        for n in range(NB):
            for e in range(E):
                nc.tensor.matmul(
                    w2ps[e][:],
                    w2_sb[:, n, e, :],
                    ones[:],
                    start=(n == 0),
                    stop=(n == NB - 1),
                )
        # w2sum as a bf16 hi+lo column pair per expert (a single-bf16 cast
        # would dominate the accuracy budget)
        w2sf = pool.tile([P, E], F32)
        w2hl = pool.tile([P, E, 2], BF16)
        w2r32 = pool.tile([P, E], F32)
        for e in range(E):
            nc.vector.tensor_copy(w2sf[:, e : e + 1], w2ps[e][:])
            nc.vector.tensor_copy(w2hl[:, e, 0:1], w2ps[e][:])
        nc.vector.tensor_copy(w2r32[:], w2hl[:, :, 0])
        w2lo = pool.tile([P, E], F32)
        nc.vector.tensor_sub(w2lo[:], w2sf[:], w2r32[:])
        nc.vector.tensor_copy(w2hl[:, :, 1], w2lo[:])

        # v[e] = W1[e]^T-stationary @ [w2sum_hi | w2sum_lo]-moving: v comes
        # out PARTITION-major ([128, NB, 2] per expert), so the PSUM->SBUF
        # hop is two wide copies, not eight single-partition crawls
        pay3 = pool.tile([P, E, NB], F32)
        for e in range(E):
            vps = psum.tile([P, NB, 2], F32, name=f"vps_{e}")
            for n in range(NB):
                nc.tensor.matmul(
                    vps[:, n, :],
                    w1_sb[:, e, n * P : (n + 1) * P],
                    w2hl[:, e, :],
                    start=True,
                    stop=True,
                )
            vt = pool.tile([P, NB, 2], F32, name=f"vt_{e}")
            nc.vector.tensor_copy(vt[:], vps[:])
            nc.vector.tensor_add(
                pay3[:, e, :, None], vt[:, :, 0:1], vt[:, :, 1:2]
            )
        nc.sync.dma_start(vout[:, 0 : E * NB], pay3[:])

        # c[e] = b1[e].w2sum[e] + sum(b2[e])   (b1/b2 are zeros per spec,
        # kept for generality; bf16 b1 path is accuracy-irrelevant here)
        b1ps = psum.tile([1, E], F32)
        for e in range(E):
            nc.tensor.matmul(
                b1ps[0:1, e : e + 1],
                w2hl[:, e, 0:1],
                b1_sb[:, e : e + 1],
                start=True,
                stop=True,
            )
        b2s = pool.tile([1, E], F32)
        for e in range(E):
            nc.vector.reduce_sum(
                b2s[0:1, e : e + 1], b2_sb[0:1, e * DC : (e + 1) * DC], axis=AX.X
            )
        cpay = pool.tile([1, E], F32)
        nc.vector.tensor_add(cpay[:], b1ps[:], b2s[:])
        nc.gpsimd.dma_start(vout[0:1, E * NB : E * NB + E], cpay[:])

        # exact xl @ [wgh|wgl] logit-correction for d-blocks 0..NSPLIT-1 of
        # this core's batch row (the host adds the 4 rows pairwise and routes
        # them to launch B) — runs on the otherwise-idle PE, chasing the xl
        # DMA chunks
        lo4 = psum.tile([4, TB], F32)
        for n in range(NSPLIT):
            nc.tensor.matmul(
                lo4[:],
                m4_sb[:, n, :],
                xl_sb[:, n, :],
                start=(n == 0),
                stop=(n == NSPLIT - 1),
            )
        lo_sb = pool.tile([4, TB], F32)
        nc.vector.tensor_copy(lo_sb[:], lo4[:])
        nc.scalar.dma_start(lo_out[:], lo_sb[:])


def emit_phase_b(nc, tc, io):
    """hi/lo bf16 logits+s stream, gate/select, row log_softmax."""
    x2, m8d, csum_d, lo8d, out = io["x2"], io["m8"], io["csum"], io["lo8"], io["out"]
    with (
        tc.tile_pool(name="main", bufs=1) as pool,
        tc.tile_pool(name="psum", bufs=1, space="PSUM") as psum,
    ):
        # m8 first (first matmul needs it), then the x blocks (xh 0..15,
        # then xl NSPLIT..15) alternating the two HWDGE rings; the last
        # chunk is kept small so the PE can finish right behind the DMA.
        m8 = pool.tile([P, NB, 8], BF16)
        nc.sync.dma_start(m8[:], m8d)
        x_sb = pool.tile([P, XB, TB], BF16)
        qs = [nc.sync, nc.scalar]
        chunks = [
            (1, 0, 1), (0, 1, 2),
            (1, 2, 4), (0, 4, 7),
            (1, 7, 10), (0, 10, 13),
            (1, 13, 16), (0, 16, 18),
            (1, 18, XB),
        ]
        for q, lo, hi in chunks:
            if lo < hi:
                qs[q].dma_start(x_sb[:, lo:hi], x2[:, lo:hi])
        csum = pool.tile([1, E], F32)
        nc.gpsimd.dma_start(csum[:], csum_d)
        lo8 = pool.tile([P, NG, E], F32)
        nc.gpsimd.dma_start(lo8[:], lo8d[:])

        # PE warm-up during the DMA window (HAM ramp; memsets on DVE whose
        # queue frees up earliest)
        dum = pool.tile([P, 512], BF16)
        nc.vector.memset(dum[:], 0.25)
        st1 = pool.tile([P, 1], BF16)
        nc.vector.memset(st1[:], 0.5)
        wps = psum.tile([1, 512], F32, name="warmps")
        for _ in range(6):
            nc.tensor.matmul(wps[:], st1[:], dum[:], start=True, stop=True)

        # preload the Sigmoid table (gate).  The table cache holds ~one
        # entry, so Exp/Ln are each prefetched just after the preceding
        # table's last use, hidden under DVE/PE work.  Reading csum (not a
        # const) delays this load until after the ring triggers, so it
        # can't head-of-line block the x DMA.
        wz = pool.tile([1, E], F32)
        nc.scalar.activation(wz[:], csum[0:1, :], AF.Sigmoid)

        ident = pool.tile([P, P], F32)
        make_identity(nc, ident[:])
        # c broadcast tile on every partition, replicated per token group
        cb8 = pool.tile([P, NG, E], F32)
        for g in range(NG):
            nc.gpsimd.partition_broadcast(cb8[:, g, :], csum[0:1, :])

        # psum [8, TB] accumulates the xh stream (all blocks) and the tail
        # xl blocks against the M=8 stationary
        # [wgh0 wgh1 wgl0 wgl1 vh0 vh1 vl0 vl1] per d-block
        ps8 = psum.tile([8, TB], F32)
        for j in range(XB):
            n = j if j < NB else NSPLIT + (j - NB)
            nc.tensor.matmul(
                ps8[:],
                m8[:, n, :],
                x_sb[:, j, :],
                start=(j == 0),
                stop=(j == XB - 1),
            )
        sbl = pool.tile([8, TB], F32)
        for g in range(NG):
            nc.vector.tensor_copy(
                sbl[0:8, g * P : (g + 1) * P], ps8[0:8, g * P : (g + 1) * P]
            )

        # token-major via 4 PE transposes into one PSUM tile, then ALL
        # gating math batched across the 4 groups in single strided DVE ops.
        # gate = softmax(l).max == sigmoid(|l0-l1|), mask = (l0 >= l1).
        tpa = psum.tile([P, NG, 8], F32)
        for g in range(NG):
            nc.tensor.transpose(
                tpa[:, g, :], sbl[0:8, g * P : (g + 1) * P], ident[0:8, 0:8]
            )
        t8a = pool.tile([P, NG, 8], F32)
        nc.vector.tensor_copy(t8a[:], tpa[:])
        l4 = pool.tile([P, NG, E], F32)
        nc.vector.tensor_add(l4[:], t8a[:, :, 0:2], t8a[:, :, 2:4])  # logits
        nc.vector.tensor_add(l4[:], l4[:], lo8[:])  # xl correction from A
        s4p = pool.tile([P, NG, E], F32)
        nc.vector.tensor_add(s4p[:], t8a[:, :, 4:6], t8a[:, :, 6:8])  # s
        nc.vector.tensor_add(s4p[:], s4p[:], cb8[:])
        dl = pool.tile([P, NG, 1], F32)
        nc.vector.tensor_sub(dl[:], l4[:, :, 0:1], l4[:, :, 1:2])
        ndl = pool.tile([P, NG, 1], F32)
        nc.vector.tensor_scalar_mul(ndl[:], dl[:], -1.0)
        absdl = pool.tile([P, NG, 1], F32)
        nc.vector.tensor_tensor(absdl[:], dl[:], ndl[:], op=ALU.max)
        gate = pool.tile([P, NG, 1], F32)
        nc.scalar.activation(gate[:], absdl[:], AF.Sigmoid)
        # prefetch the Exp table (for the row softmax) while DVE finishes
        # the select math
        nc.scalar.activation(wz[:], csum[0:1, :], AF.Exp)
        mask = pool.tile([P, NG, 1], F32)
        nc.vector.tensor_scalar(mask[:], dl[:], 0.0, None, op0=ALU.is_ge)
        sdiff = pool.tile([P, NG, 1], F32)
        nc.vector.tensor_sub(sdiff[:], s4p[:, :, 0:1], s4p[:, :, 1:2])
        ssel = pool.tile([P, NG, 1], F32)
        nc.vector.tensor_mul(ssel[:], mask[:], sdiff[:])
        nc.vector.tensor_add(ssel[:], ssel[:], s4p[:, :, 1:2])
        moe_sb = pool.tile([P, NG], F32)
        nc.vector.tensor_mul(moe_sb[:, :, None], gate[:], ssel[:])

        # row log_softmax over all 512 tokens, via PE transposes
        tp4 = psum.tile([NG, P], F32)
        nc.tensor.transpose(tp4[:], moe_sb[:], ident[:])
        sb4t = pool.tile([NG, P], F32)
        nc.vector.tensor_copy(sb4t[:], tp4[:])
        m4p = pool.tile([NG, 1], F32)
        nc.vector.reduce_max(m4p[:], sb4t[:], axis=AX.X)
        m1p = psum.tile([1, NG], F32, name="m1p", tag="t1", bufs=2)
        nc.tensor.transpose(m1p[:], m4p[:], ident[0:NG, 0:NG])
        negm2 = pool.tile([1, 1], F32)
        nc.vector.reduce_max(negm2[:], m1p[:], axis=AX.X, negate=True)
        negm4 = pool.tile([NG, 1], F32)
        nc.gpsimd.partition_broadcast(negm4[:], negm2[:])
        e4 = pool.tile([NG, P], F32)
        s4 = pool.tile([NG, 1], F32)
        nc.scalar.activation(e4[:], sb4t[:], AF.Exp, bias=negm4[:], accum_out=s4[:])
        # load the Ln table NOW so the real Ln below table-hits; overlaps
        # the transpose+reduce running on other engines
        nc.scalar.activation(wz[:], csum[0:1, :], AF.Ln)
        s1p = psum.tile([1, NG], F32, name="s1p", tag="t1", bufs=2)
        nc.tensor.transpose(s1p[:], s4[:], ident[0:NG, 0:NG])
        ssum = pool.tile([1, 1], F32)
        nc.vector.reduce_sum(ssum[:], s1p[:], axis=AX.X)
        logs = pool.tile([1, 1], F32)
        nc.scalar.activation(logs[:], ssum[:], AF.Ln)
        shift = pool.tile([1, 1], F32)
        nc.vector.tensor_sub(shift[:], negm2[:], logs[:])
        shift4 = pool.tile([NG, 1], F32)
        nc.gpsimd.partition_broadcast(shift4[:], shift[:])
        res4 = pool.tile([NG, P], F32)
        nc.vector.tensor_scalar_add(res4[:], sb4t[:], shift4[:])
        nc.sync.dma_start(out.rearrange("x (g p) -> g (x p)", p=P), res4[:])


_CACHED = {}


def build_program(which):
    if which in _CACHED:
        return _CACHED[which]
    nc = bacc.Bacc(
        "TRN2",
        target_bir_lowering=False,
        debug=False,
        enable_asserts=False,
        num_devices=NCORES,
    )
    if which == "a":
        io = {
            "w2d": nc.dram_tensor("w2d", [P, NB, E, HC], BF16, kind="ExternalInput").ap(),
            "w1t": nc.dram_tensor("w1t", [P, E, D], BF16, kind="ExternalInput").ap(),
            "b1t": nc.dram_tensor("b1t", [P, E], BF16, kind="ExternalInput").ap(),
            "b2c": nc.dram_tensor("b2c", [1, E * DC], F32, kind="ExternalInput").ap(),
            "xlr": nc.dram_tensor("xlr", [P, NSPLIT, TB], BF16, kind="ExternalInput").ap(),
            "m4a": nc.dram_tensor("m4a", [P, NSPLIT, 4], BF16, kind="ExternalInput").ap(),
            "vout": nc.dram_tensor("vout", [P, VCOLS], F32, kind="ExternalOutput").ap(),
            "lo_out": nc.dram_tensor("lo_out", [4, TB], F32, kind="ExternalOutput").ap(),
        }
        emit = emit_phase_a
    else:
        io = {
            "x2": nc.dram_tensor("x2", [P, XB, TB], BF16, kind="ExternalInput").ap(),
            "m8": nc.dram_tensor("m8", [P, NB, 8], BF16, kind="ExternalInput").ap(),
            "csum": nc.dram_tensor("csum", [1, E], F32, kind="ExternalInput").ap(),
            "lo8": nc.dram_tensor("lo8", [P, NG, E], F32, kind="ExternalInput").ap(),
            "out": nc.dram_tensor("out", [1, TB], F32, kind="ExternalOutput").ap(),
        }
        emit = emit_phase_b
    with tile.TileContext(nc) as tc:
        emit(nc, tc, io)
    nc.compile()
    _CACHED[which] = nc
    return nc


def _hi_lo(a):
    import ml_dtypes

    hi = a.astype(ml_dtypes.bfloat16)
    lo = (a - hi.astype(np.float32)).astype(ml_dtypes.bfloat16)
    return hi, lo


def shard_inputs_a(Wg, W1, b1, W2, b2, x):
    import ml_dtypes

    bf16 = ml_dtypes.bfloat16
    Wg = np.asarray(Wg, np.float32)
    W1 = np.asarray(W1, np.float32)
    b1 = np.asarray(b1, np.float32)
    W2 = np.asarray(W2, np.float32)
    b2 = np.asarray(b2, np.float32)
    x = np.asarray(x, np.float32).reshape(B * T, D)
    # m4a[p, n, :] = [wgh0 wgh1 wgl0 wgl1] at d = n*128 + p, n < NSPLIT
    wgh, wgl = _hi_lo(Wg)  # [D, E]
    m4 = np.concatenate([wgh, wgl], axis=1)  # [D, 4]
    m4a = np.ascontiguousarray(m4.reshape(NB, P, 4).transpose(1, 0, 2)[:, :NSPLIT])
    in_maps = []
    for c in range(NCORES):
        hs, he = c * HC, (c + 1) * HC
        # w2d[p, n, e, h] = W2[e, hs+h, p*16+n]  (d = p*16 + n: 8KB runs)
        w2d = np.ascontiguousarray(
            W2[:, hs:he, :].transpose(2, 0, 1).reshape(P, NB, E, HC).astype(bf16)
        )
        # w1t[h, e, d] = W1[e, d, hs+h]
        w1t = np.ascontiguousarray(W1[:, :, hs:he].transpose(2, 0, 1).astype(bf16))
        b1t = np.ascontiguousarray(b1[:, hs:he].T.astype(bf16))
        # xl residual of this core's batch row, d-blocks 0..NSPLIT-1
        row = c % B
        _, xl = _hi_lo(x[row * TB : (row + 1) * TB, :].T)  # [D, TB]
        xlr = np.ascontiguousarray(
            np.asarray(xl).reshape(NB, P, TB).transpose(1, 0, 2)[:, :NSPLIT]
        )
        in_maps.append(
            {
                "w2d": w2d,
                "w1t": w1t,
                "b1t": b1t,
                "b2c": np.ascontiguousarray(
                    b2[:, c * DC : (c + 1) * DC].reshape(1, E * DC)
                ),
                "xlr": xlr,
                "m4a": m4a,
            }
        )
    return in_maps


def shard_inputs_b(x, Wg, vpart_sum, lo_rows):
    x = np.asarray(x, np.float32).reshape(B * T, D)
    Wg = np.asarray(Wg, np.float32)
    arr = np.asarray(vpart_sum, np.float32).reshape(P, VCOLS)
    vm = arr[:, : E * NB].reshape(P, E, NB)
    # v[e, n*128+p] = vm[p, e, n]
    v = np.stack([vm[:, e, :].T.reshape(-1) for e in range(E)])  # [E, D]
    csum = np.ascontiguousarray(arr[0:1, E * NB : E * NB + E])
    # m8[p, n, :] = [wgh0 wgh1 wgl0 wgl1 vh0 vh1 vl0 vl1] at d = n*128 + p
    wgh, wgl = _hi_lo(Wg)  # [D, E]
    vh, vl = _hi_lo(v.T)  # [D, E]
    m8 = np.concatenate([wgh, wgl, vh, vl], axis=1)  # [D, 8]
    m8 = np.ascontiguousarray(m8.reshape(NB, P, 8).transpose(1, 0, 2))
    # lo_rows[r] is launch A's [4, TB] xl@[wgh|wgl] partial for batch row r;
    # pairwise row-sum -> per-token logit correction, token-major [P, NG, E]
    lo_rows = np.asarray(lo_rows, np.float32)  # [B, 4, TB]
    lo = lo_rows[:, 0:2, :] + lo_rows[:, 2:4, :]  # [B, E, TB]
    in_maps = []
    for c in range(NCORES):
        row = c % B
        xr = x[row * TB : (row + 1) * TB, :]  # [TB, D]
        xh, xl = _hi_lo(xr.T)  # [D, TB]
        xh3 = np.asarray(xh).reshape(NB, P, TB)
        xl3 = np.asarray(xl).reshape(NB, P, TB)
        # x2[p, j, t]: xh blocks 0..15 then xl blocks NSPLIT..15
        x2 = np.ascontiguousarray(
            np.concatenate([xh3, xl3[NSPLIT:]], axis=0).transpose(1, 0, 2)
        )
        lo8 = np.ascontiguousarray(
            lo[row].T.reshape(NG, P, E).transpose(1, 0, 2)
        )  # lo8[p, g, e] = lo[row, e, g*128+p]
        in_maps.append({"x2": x2, "m8": m8, "csum": csum, "lo8": lo8})
    return in_maps


def run_a(in_maps, **kwargs):
    return bass_utils.run_bass_kernel_spmd(
        build_program("a"), in_maps, core_ids=list(range(NCORES)), **kwargs
    )


def run_b(in_maps, **kwargs):
    return bass_utils.run_bass_kernel_spmd(
        build_program("b"), in_maps, core_ids=list(range(NCORES)), **kwargs
    )


def kernel(x, Wg, W1, b1, W2, b2):
    res_a = run_a(shard_inputs_a(Wg, W1, b1, W2, b2, x))
    # cross-core combine: sum of the 8 per-core v/c partials and gather of
    # the per-row xl logit partials (the reshard step between the launches;
    # ~24KB, no model math beyond the partial-sum reductions)
    vpart = np.sum([res_a.results[c]["vout"] for c in range(NCORES)], axis=0)
    vpart = np.ascontiguousarray(vpart, np.float32)
    lo_rows = np.stack([res_a.results[r]["lo_out"] for r in range(B)])
    res_b = run_b(shard_inputs_b(x, Wg, vpart, lo_rows))
    return np.concatenate([res_b.results[b]["out"] for b in range(B)], axis=0)


# revision 39
# speedup vs baseline: 1.4355x; 1.0606x over previous
"""Trainium2 Bass kernel for nn_ExampleModel_1116691497724 (moe_routing).

Math: the reference returns log_softmax_T( sum_D(moe_out) ), and sum_D
collapses the expert FFN to a dot product:
    sum_d (h @ W2[e] + b2[e]) = h . w2sum[e] + sum(b2[e]),  w2sum[e] = W2[e] @ 1
    (x @ W1[e] + b1[e]) . w2sum[e] = x . v[e] + c[e]
with v[e] = W1[e] @ w2sum[e]  (a [D] vector) and scalar
c[e] = b1[e].w2sum[e] + sum(b2[e]).  Then per token:
    s_e = x . v[e] + c[e],  logits = x @ Wg
    moe_sum = max(softmax(logits)) * s_argmax(logits)
    out = log_softmax over tokens (per batch row) of moe_sum.

Distribution over 8 cores, two launches (an on-device ncfw collective costs
~65us of barrier/trigger latency on this runtime, far more than a second
launch; the 16KB cross-core combine of v-partials happens on the host between
launches — the host does only that partial sum, all real math stays on device):
  launch A (expert-parallel over H): core c owns h-chunk [128c,128c+128) of
    both experts.  W2 ships bf16 d-major so w2sum is a PE ones-matmul
    (stationary [128d,128h] tiles, FWL bf16 loads), W1 ships bf16 h-major so
    v = w2sum^T-stationary @ W1-moving streams at 1 cyc/row.  Outputs
    [v0 | v1 | c0 c1] partials (16KB); host sums the 8 payloads.
  launch B (token-parallel): core c owns batch row c%4 (512 tokens).  x ships
    as a bf16 hi/lo pair (x = xh + xl exactly to ~2^-17), and one M=8
    stationary [wgh0 wgh1 wgl0 wgl1 vh0 vh1 vl0 vl1] (bf16 hi/lo of Wg and v)
    is streamed by xh then xl at 1 cyc/row: all four cross products accumulate
    in fp32 PSUM, so logits are fp32-grade (argmax must match the reference;
    bf16-only logits would flip near-boundary tokens) while the whole PE
    stream is 4x cheaper than an fp32 x stream.  l_e = col_e+col_{2+e},
    s_e = col_{4+e}+col_{6+e}+c_e after a PE transpose to token-major; then
    gate/select per token and the row log_softmax via PE transposes exactly
    as before (no cross-partition DMA).  Host takes rows from cores 0..3.

Scheduling: both launches issue the big HBM loads on the two HWDGE rings
(SP via nc.sync, ACT via nc.scalar) as their first instructions, before any
ACT-table load can head-of-line block a ring.  All hi/lo splits, transposes
and packing happen on the host (input reformatting only).
"""

import sys

import numpy as np

for _p in ("/opt/trn_rl_repo",):
    if _p not in sys.path:
        sys.path.append(_p)

import concourse.bass as bass  # noqa: E402
import concourse.mybir as mybir  # noqa: E402
import concourse.tile as tile  # noqa: E402
from concourse import bacc, bass_utils  # noqa: E402
from concourse.masks import make_identity  # noqa: E402

# Problem shape (hardcoded per spec).
B, T, D, H, E = 4, 512, 2048, 1024, 2
P = 128
NCORES = 8
TB = T  # tokens per core = one batch row
NB = D // P  # 16 d-blocks
HC = H // NCORES  # 128 h-chunk per expert per core
NG = TB // P  # 4 token groups per core
DC = D // NCORES  # 256 b2 columns per core
VK = 4  # v computed in VK chunks of D/VK columns
NSPLIT = 14  # xl d-blocks 0..NSPLIT-1 stream in launch A, the rest in B
NL = NSPLIT // 2  # each A core streams half its row's xl blocks (pair-split)
XB = NB + (NB - NSPLIT)  # moving blocks in launch B: xh 0..15 then xl NSPLIT..15
F32 = mybir.dt.float32
BF16 = mybir.dt.bfloat16
AX = mybir.AxisListType
AF = mybir.ActivationFunctionType
ALU = mybir.AluOpType

# launch A output: [128, E*NB + E] f32 — v partition-major (col e*NB+n on
# partition p holds v[e, n*128+p]) plus c0,c1 on partition 0
VCOLS = E * NB + E


def emit_phase_a(nc, tc, io):
    """w2sum (PE ones-matmul) + partial v for this core's H-chunk."""
    w2d, w1t, b1t, b2c = io["w2d"], io["w1t"], io["b1t"], io["b2c"]
    xlr, m4a, vout, lo_out = io["xlr"], io["m4a"], io["vout"], io["lo_out"]
    with (
        tc.tile_pool(name="main", bufs=1) as pool,
        tc.tile_pool(name="psum", bufs=1, space="PSUM") as psum,
    ):
        # Big loads first on both HWDGE rings, balanced ~1.75MB each.  W2
        # (d-major) gates the reduce so it leads ring 0; W1 per-expert leads
        # ring 1 so the v-chain starts early; the xl halves trail both rings.
        HS = 3  # xl blocks 0..2 ride ring 0 behind W2; the rest ring 1
        w2_sb = pool.tile([P, NB, E, HC], BF16)
        nc.sync.dma_start(w2_sb[:], w2d[:])
        w1_sb = pool.tile([P, E, D], BF16)
        nc.scalar.dma_start(w1_sb[:, 0, :], w1t[:, 0, :])
        nc.scalar.dma_start(w1_sb[:, 1, :], w1t[:, 1, :])
        xl_sb = pool.tile([P, NL, TB], BF16)
        nc.sync.dma_start(xl_sb[:, 0:HS], xlr[:, 0:HS])
        nc.scalar.dma_start(xl_sb[:, HS : NL - 1], xlr[:, HS : NL - 1])
        nc.scalar.dma_start(xl_sb[:, NL - 1 : NL], xlr[:, NL - 1 : NL])
        b1_sb = pool.tile([P, E], BF16)
        nc.gpsimd.dma_start(b1_sb[:], b1t)
        b2_sb = pool.tile([1, E * DC], F32)
        nc.gpsimd.dma_start(b2_sb[:], b2c)
        m4_sb = pool.tile([P, NL, 4], BF16)
        nc.gpsimd.dma_start(m4_sb[:], m4a[:])

        ones = pool.tile([P, 1], BF16)
        nc.vector.memset(ones[:], 1.0)

        # PE warm-up during the DMA window: sustained dummy matmuls ramp the
        # HAM clock so the real streams run fast (memsets on DVE, whose
        # queue frees up earliest)
        dum = pool.tile([P, 512], BF16)
        nc.vector.memset(dum[:], 0.25)
        wps = psum.tile([1, 512], F32, name="warmps")
        for _ in range(6):
            nc.tensor.matmul(wps[:], ones[:], dum[:], start=True, stop=True)

        # w2sum[e, h] via PE: stationary [128d, 128h] tiles, moving ones.
        w2ps = [psum.tile([P, 1], F32, name=f"w2ps_{e}") for e in range(E)]
        for n in range(NB):
            for e in range(E):
                nc.tensor.matmul(
                    w2ps[e][:],
                    w2_sb[:, n, e, :],
                    ones[:],
                    start=(n == 0),
                    stop=(n == NB - 1),
                )
        # first lo-correction blocks (chasing ring 0) keep the PE busy while
        # DVE builds w2hl below
        lo4 = psum.tile([4, TB], F32)
        for n in range(HS):
            nc.tensor.matmul(
                lo4[:], m4_sb[:, n, :], xl_sb[:, n, :], start=(n == 0), stop=False
            )

        # w2sum as a bf16 hi+lo column pair per expert (a single-bf16 cast
        # would dominate the accuracy budget)
        w2sf = pool.tile([P, E], F32)
        w2hl = pool.tile([P, E, 2], BF16)
        w2r32 = pool.tile([P, E], F32)
        for e in range(E):
            nc.vector.tensor_copy(w2sf[:, e : e + 1], w2ps[e][:])
            nc.vector.tensor_copy(w2hl[:, e, 0:1], w2ps[e][:])
        nc.vector.tensor_copy(w2r32[:], w2hl[:, :, 0])
        w2lo = pool.tile([P, E], F32)
        nc.vector.tensor_sub(w2lo[:], w2sf[:], w2r32[:])
        nc.vector.tensor_copy(w2hl[:, :, 1], w2lo[:])

        # v[e] = W1[e]^T-stationary @ [w2sum_hi | w2sum_lo]-moving: v comes
        # out PARTITION-major ([128, NB, 2] per expert), so the PSUM->SBUF
        # hop is two wide copies, not eight single-partition crawls
        pay3 = pool.tile([P, E, NB], F32)
        for e in range(E):
            vps = psum.tile([P, NB, 2], F32, name=f"vps_{e}")
            for n in range(NB):
                nc.tensor.matmul(
                    vps[:, n, :],
                    w1_sb[:, e, n * P : (n + 1) * P],
                    w2hl[:, e, :],
                    start=True,
                    stop=True,
                )
            vt = pool.tile([P, NB, 2], F32, name=f"vt_{e}")
            nc.vector.tensor_copy(vt[:], vps[:])
            nc.vector.tensor_add(
                pay3[:, e, :, None], vt[:, :, 0:1], vt[:, :, 1:2]
            )
        nc.sync.dma_start(vout[:, 0 : E * NB], pay3[:])

        # c[e] = b1[e].w2sum[e] + sum(b2[e])   (b1/b2 are zeros per spec,
        # kept for generality; bf16 b1 path is accuracy-irrelevant here)
        b1ps = psum.tile([1, E], F32)
        for e in range(E):
            nc.tensor.matmul(
                b1ps[0:1, e : e + 1],
                w2hl[:, e, 0:1],
                b1_sb[:, e : e + 1],
                start=True,
                stop=True,
            )
        b2s = pool.tile([1, E], F32)
        for e in range(E):
            nc.vector.reduce_sum(
                b2s[0:1, e : e + 1], b2_sb[0:1, e * DC : (e + 1) * DC], axis=AX.X
            )
        cpay = pool.tile([1, E], F32)
        nc.vector.tensor_add(cpay[:], b1ps[:], b2s[:])
        nc.gpsimd.dma_start(vout[0:1, E * NB : E * NB + E], cpay[:])

        # remaining exact xl @ [wgh|wgl] lo-correction blocks for this
        # core's half of its batch row (cores c and c+4 split the row's
        # blocks; the host sums the two partials and routes them to B)
        for n in range(HS, NL):
            nc.tensor.matmul(
                lo4[:],
                m4_sb[:, n, :],
                xl_sb[:, n, :],
                start=False,
                stop=(n == NL - 1),
            )
        lo_sb = pool.tile([4, TB], F32)
        nc.vector.tensor_copy(lo_sb[:], lo4[:])
        nc.scalar.dma_start(lo_out[:], lo_sb[:])


def emit_phase_b(nc, tc, io):
    """hi/lo bf16 logits+s stream, gate/select, row log_softmax."""
    x2, m8d, csum_d, lo8d, out = io["x2"], io["m8"], io["csum"], io["lo8"], io["out"]
    with (
        tc.tile_pool(name="main", bufs=1) as pool,
        tc.tile_pool(name="psum", bufs=1, space="PSUM") as psum,
    ):
        # m8 first (first matmul needs it), then the x blocks (xh 0..15,
        # then xl NSPLIT..15) alternating the two HWDGE rings; the last
        # chunk is kept small so the PE can finish right behind the DMA.
        m8 = pool.tile([P, NB, 8], BF16)
        nc.sync.dma_start(m8[:], m8d)
        x_sb = pool.tile([P, XB, TB], BF16)
        qs = [nc.sync, nc.scalar]
        chunks = [
            (1, 0, 1), (0, 1, 3),
            (1, 3, 6), (0, 6, 10),
            (1, 10, 14), (0, 14, 18),
            (1, 18, XB),
        ]
        for q, lo, hi in chunks:
            if lo < hi:
                qs[q].dma_start(x_sb[:, lo:hi], x2[:, lo:hi])
        csum = pool.tile([1, E], F32)
        nc.gpsimd.dma_start(csum[:], csum_d)
        lo8 = pool.tile([P, NG, E], F32)
        nc.gpsimd.dma_start(lo8[:], lo8d[:])

        # PE warm-up during the DMA window (HAM ramp; memsets on DVE whose
        # queue frees up earliest)
        dum = pool.tile([P, 512], BF16)
        nc.vector.memset(dum[:], 0.25)
        st1 = pool.tile([P, 1], BF16)
        nc.vector.memset(st1[:], 0.5)
        wps = psum.tile([1, 512], F32, name="warmps")
        for _ in range(6):
            nc.tensor.matmul(wps[:], st1[:], dum[:], start=True, stop=True)

        # preload the Sigmoid table (gate).  The table cache holds ~one
        # entry, so Exp/Ln are each prefetched just after the preceding
        # table's last use, hidden under DVE/PE work.  Reading csum (not a
        # const) delays this load until after the ring triggers, so it
        # can't head-of-line block the x DMA.
        wz = pool.tile([1, E], F32)
        nc.scalar.activation(wz[:], csum[0:1, :], AF.Sigmoid)

        ident = pool.tile([P, P], F32)
        make_identity(nc, ident[:])
        # c broadcast tile on every partition, replicated per token group
        cb8 = pool.tile([P, NG, E], F32)
        for g in range(NG):
            nc.gpsimd.partition_broadcast(cb8[:, g, :], csum[0:1, :])

        # psum [8, TB] accumulates the xh stream (all blocks) and the tail
        # xl blocks against the M=8 stationary
        # [wgh0 wgh1 wgl0 wgl1 vh0 vh1 vl0 vl1] per d-block
        ps8 = psum.tile([8, TB], F32)
        for j in range(XB):
            n = j if j < NB else NSPLIT + (j - NB)
            nc.tensor.matmul(
                ps8[:],
                m8[:, n, :],
                x_sb[:, j, :],
                start=(j == 0),
                stop=(j == XB - 1),
            )
        sbl = pool.tile([8, TB], F32)
        for g in range(NG):
            nc.vector.tensor_copy(
                sbl[0:8, g * P : (g + 1) * P], ps8[0:8, g * P : (g + 1) * P]
            )

        # token-major via 4 PE transposes into one PSUM tile, then ALL
        # gating math batched across the 4 groups in single strided DVE ops.
        # gate = softmax(l).max == sigmoid(|l0-l1|), mask = (l0 >= l1).
        tpa = psum.tile([P, NG, 8], F32)
        for g in range(NG):
            nc.tensor.transpose(
                tpa[:, g, :], sbl[0:8, g * P : (g + 1) * P], ident[0:8, 0:8]
            )
        t8a = pool.tile([P, NG, 8], F32)
        nc.vector.tensor_copy(t8a[:], tpa[:])
        l4 = pool.tile([P, NG, E], F32)
        nc.vector.tensor_add(l4[:], t8a[:, :, 0:2], t8a[:, :, 2:4])  # logits
        nc.vector.tensor_add(l4[:], l4[:], lo8[:])  # xl correction from A
        s4p = pool.tile([P, NG, E], F32)
        nc.vector.tensor_add(s4p[:], t8a[:, :, 4:6], t8a[:, :, 6:8])  # s
        nc.vector.tensor_add(s4p[:], s4p[:], cb8[:])
        dl = pool.tile([P, NG, 1], F32)
        nc.vector.tensor_sub(dl[:], l4[:, :, 0:1], l4[:, :, 1:2])
        ndl = pool.tile([P, NG, 1], F32)
        nc.vector.tensor_scalar_mul(ndl[:], dl[:], -1.0)
        absdl = pool.tile([P, NG, 1], F32)
        nc.vector.tensor_tensor(absdl[:], dl[:], ndl[:], op=ALU.max)
        gate = pool.tile([P, NG, 1], F32)
        nc.scalar.activation(gate[:], absdl[:], AF.Sigmoid)
        # prefetch the Exp table (for the row softmax) while DVE finishes
        # the select math.  Input reads `gate` so the Tile scheduler cannot
        # hoist this load ahead of the Sigmoid use (tables thrash otherwise).
        wze = pool.tile([1, 1], F32)
        nc.scalar.activation(wze[:], gate[0:1, 0, :], AF.Exp)
        mask = pool.tile([P, NG, 1], F32)
        nc.vector.tensor_scalar(mask[:], dl[:], 0.0, None, op0=ALU.is_ge)
        sdiff = pool.tile([P, NG, 1], F32)
        nc.vector.tensor_sub(sdiff[:], s4p[:, :, 0:1], s4p[:, :, 1:2])
        ssel = pool.tile([P, NG, 1], F32)
        nc.vector.tensor_mul(ssel[:], mask[:], sdiff[:])
        nc.vector.tensor_add(ssel[:], ssel[:], s4p[:, :, 1:2])
        moe_sb = pool.tile([P, NG], F32)
        nc.vector.tensor_mul(moe_sb[:, :, None], gate[:], ssel[:])

        # row log_softmax over all 512 tokens, via PE transposes
        tp4 = psum.tile([NG, P], F32)
        nc.tensor.transpose(tp4[:], moe_sb[:], ident[:])
        sb4t = pool.tile([NG, P], F32)
        nc.vector.tensor_copy(sb4t[:], tp4[:])
        m4p = pool.tile([NG, 1], F32)
        nc.vector.reduce_max(m4p[:], sb4t[:], axis=AX.X)
        m1p = psum.tile([1, NG], F32, name="m1p", tag="t1", bufs=2)
        nc.tensor.transpose(m1p[:], m4p[:], ident[0:NG, 0:NG])
        negm2 = pool.tile([1, 1], F32)
        nc.vector.reduce_max(negm2[:], m1p[:], axis=AX.X, negate=True)
        negm4 = pool.tile([NG, 1], F32)
        nc.gpsimd.partition_broadcast(negm4[:], negm2[:])
        e4 = pool.tile([NG, P], F32)
        s4 = pool.tile([NG, 1], F32)
        nc.scalar.activation(e4[:], sb4t[:], AF.Exp, bias=negm4[:], accum_out=s4[:])
        # load the Ln table NOW so the real Ln below table-hits; overlaps
        # the transpose+reduce running on other engines.  Input reads e4 to
        # pin this load after the row-Exp (scheduler ordering).
        wzl = pool.tile([1, 1], F32)
        nc.scalar.activation(wzl[:], e4[0:1, 0:1], AF.Ln)
        s1p = psum.tile([1, NG], F32, name="s1p", tag="t1", bufs=2)
        nc.tensor.transpose(s1p[:], s4[:], ident[0:NG, 0:NG])
        ssum = pool.tile([1, 1], F32)
        nc.vector.reduce_sum(ssum[:], s1p[:], axis=AX.X)
        logs = pool.tile([1, 1], F32)
        nc.scalar.activation(logs[:], ssum[:], AF.Ln)
        shift = pool.tile([1, 1], F32)
        nc.vector.tensor_sub(shift[:], negm2[:], logs[:])
        shift4 = pool.tile([NG, 1], F32)
        nc.gpsimd.partition_broadcast(shift4[:], shift[:])
        res4 = pool.tile([NG, P], F32)
        nc.vector.tensor_scalar_add(res4[:], sb4t[:], shift4[:])
        nc.sync.dma_start(out.rearrange("x (g p) -> g (x p)", p=P), res4[:])


_CACHED = {}


def build_program(which):
    if which in _CACHED:
        return _CACHED[which]
    nc = bacc.Bacc(
        "TRN2",
        target_bir_lowering=False,
        debug=False,
        enable_asserts=False,
        num_devices=NCORES,
    )
    if which == "a":
        io = {
            "w2d": nc.dram_tensor("w2d", [P, NB, E, HC], BF16, kind="ExternalInput").ap(),
            "w1t": nc.dram_tensor("w1t", [P, E, D], BF16, kind="ExternalInput").ap(),
            "b1t": nc.dram_tensor("b1t", [P, E], BF16, kind="ExternalInput").ap(),
            "b2c": nc.dram_tensor("b2c", [1, E * DC], F32, kind="ExternalInput").ap(),
            "xlr": nc.dram_tensor("xlr", [P, NL, TB], BF16, kind="ExternalInput").ap(),
            "m4a": nc.dram_tensor("m4a", [P, NL, 4], BF16, kind="ExternalInput").ap(),
            "vout": nc.dram_tensor("vout", [P, VCOLS], F32, kind="ExternalOutput").ap(),
            "lo_out": nc.dram_tensor("lo_out", [4, TB], F32, kind="ExternalOutput").ap(),
        }
        emit = emit_phase_a
    else:
        io = {
            "x2": nc.dram_tensor("x2", [P, XB, TB], BF16, kind="ExternalInput").ap(),
            "m8": nc.dram_tensor("m8", [P, NB, 8], BF16, kind="ExternalInput").ap(),
            "csum": nc.dram_tensor("csum", [1, E], F32, kind="ExternalInput").ap(),
            "lo8": nc.dram_tensor("lo8", [P, NG, E], F32, kind="ExternalInput").ap(),
            "out": nc.dram_tensor("out", [1, TB], F32, kind="ExternalOutput").ap(),
        }
        emit = emit_phase_b
    with tile.TileContext(nc) as tc:
        emit(nc, tc, io)
    nc.compile()
    _CACHED[which] = nc
    return nc


def _hi_lo(a):
    import ml_dtypes

    hi = a.astype(ml_dtypes.bfloat16)
    lo = (a - hi.astype(np.float32)).astype(ml_dtypes.bfloat16)
    return hi, lo


def shard_inputs_a(Wg, W1, b1, W2, b2, x):
    import ml_dtypes

    bf16 = ml_dtypes.bfloat16
    Wg = np.asarray(Wg, np.float32)
    W1 = np.asarray(W1, np.float32)
    b1 = np.asarray(b1, np.float32)
    W2 = np.asarray(W2, np.float32)
    b2 = np.asarray(b2, np.float32)
    x = np.asarray(x, np.float32).reshape(B * T, D)
    # m4a[p, n, :] = [wgh0 wgh1 wgl0 wgl1] at d = n*128 + p
    wgh, wgl = _hi_lo(Wg)  # [D, E]
    m4 = np.concatenate([wgh, wgl], axis=1)  # [D, 4]
    m4full = m4.reshape(NB, P, 4).transpose(1, 0, 2)  # [P, NB, 4]
    in_maps = []
    for c in range(NCORES):
        hs, he = c * HC, (c + 1) * HC
        # w2d[p, n, e, h] = W2[e, hs+h, p*16+n]  (d = p*16 + n: 8KB runs)
        w2d = np.ascontiguousarray(
            W2[:, hs:he, :].transpose(2, 0, 1).reshape(P, NB, E, HC).astype(bf16)
        )
        # w1t[h, e, d] = W1[e, d, hs+h]
        w1t = np.ascontiguousarray(W1[:, :, hs:he].transpose(2, 0, 1).astype(bf16))
        b1t = np.ascontiguousarray(b1[:, hs:he].T.astype(bf16))
        # xl residual of this core's batch row: cores c and c+4 split the
        # row's NSPLIT lo-blocks in half (host sums the two lo4 partials)
        row = c % B
        off = 0 if c < B else NL
        _, xl = _hi_lo(x[row * TB : (row + 1) * TB, :].T)  # [D, TB]
        xlr = np.ascontiguousarray(
            np.asarray(xl).reshape(NB, P, TB).transpose(1, 0, 2)[:, off : off + NL]
        )
        m4a = np.ascontiguousarray(m4full[:, off : off + NL])
        in_maps.append(
            {
                "w2d": w2d,
                "w1t": w1t,
                "b1t": b1t,
                "b2c": np.ascontiguousarray(
                    b2[:, c * DC : (c + 1) * DC].reshape(1, E * DC)
                ),
                "xlr": xlr,
                "m4a": m4a,
            }
        )
    return in_maps


def shard_inputs_b(x, Wg, vpart_sum, lo_rows):
    x = np.asarray(x, np.float32).reshape(B * T, D)
    Wg = np.asarray(Wg, np.float32)
    arr = np.asarray(vpart_sum, np.float32).reshape(P, VCOLS)
    vm = arr[:, : E * NB].reshape(P, E, NB)
    # v[e, n*128+p] = vm[p, e, n]
    v = np.stack([vm[:, e, :].T.reshape(-1) for e in range(E)])  # [E, D]
    csum = np.ascontiguousarray(arr[0:1, E * NB : E * NB + E])
    # m8[p, n, :] = [wgh0 wgh1 wgl0 wgl1 vh0 vh1 vl0 vl1] at d = n*128 + p
    wgh, wgl = _hi_lo(Wg)  # [D, E]
    vh, vl = _hi_lo(v.T)  # [D, E]
    m8 = np.concatenate([wgh, wgl, vh, vl], axis=1)  # [D, 8]
    m8 = np.ascontiguousarray(m8.reshape(NB, P, 8).transpose(1, 0, 2))
    # lo_rows[r] is launch A's [4, TB] xl@[wgh|wgl] partial for batch row r;
    # pairwise row-sum -> per-token logit correction, token-major [P, NG, E]
    lo_rows = np.asarray(lo_rows, np.float32)  # [B, 4, TB]
    lo = lo_rows[:, 0:2, :] + lo_rows[:, 2:4, :]  # [B, E, TB]
    in_maps = []
    for c in range(NCORES):
        row = c % B
        xr = x[row * TB : (row + 1) * TB, :]  # [TB, D]
        xh, xl = _hi_lo(xr.T)  # [D, TB]
        xh3 = np.asarray(xh).reshape(NB, P, TB)
        xl3 = np.asarray(xl).reshape(NB, P, TB)
        # x2[p, j, t]: xh blocks 0..15 then xl blocks NSPLIT..15
        x2 = np.ascontiguousarray(
            np.concatenate([xh3, xl3[NSPLIT:]], axis=0).transpose(1, 0, 2)
        )
        lo8 = np.ascontiguousarray(
            lo[row].T.reshape(NG, P, E).transpose(1, 0, 2)
        )  # lo8[p, g, e] = lo[row, e, g*128+p]
        in_maps.append({"x2": x2, "m8": m8, "csum": csum, "lo8": lo8})
    return in_maps


def run_a(in_maps, **kwargs):
    return bass_utils.run_bass_kernel_spmd(
        build_program("a"), in_maps, core_ids=list(range(NCORES)), **kwargs
    )


def run_b(in_maps, **kwargs):
    return bass_utils.run_bass_kernel_spmd(
        build_program("b"), in_maps, core_ids=list(range(NCORES)), **kwargs
    )


def kernel(x, Wg, W1, b1, W2, b2):
    res_a = run_a(shard_inputs_a(Wg, W1, b1, W2, b2, x))
    # cross-core combine: sum of the 8 per-core v/c partials and gather of
    # the per-row xl logit partials (the reshard step between the launches;
    # ~24KB, no model math beyond the partial-sum reductions)
    vpart = np.sum([res_a.results[c]["vout"] for c in range(NCORES)], axis=0)
    vpart = np.ascontiguousarray(vpart, np.float32)
    lo_rows = np.stack(
        [res_a.results[r]["lo_out"] + res_a.results[r + B]["lo_out"] for r in range(B)]
    )
    res_b = run_b(shard_inputs_b(x, Wg, vpart, lo_rows))
    return np.concatenate([res_b.results[b]["out"] for b in range(B)], axis=0)


# revision 40
# speedup vs baseline: 1.4508x; 1.0107x over previous
"""Trainium2 Bass kernel for nn_ExampleModel_1116691497724 (moe_routing).

Math: the reference returns log_softmax_T( sum_D(moe_out) ), and sum_D
collapses the expert FFN to a dot product:
    sum_d (h @ W2[e] + b2[e]) = h . w2sum[e] + sum(b2[e]),  w2sum[e] = W2[e] @ 1
    (x @ W1[e] + b1[e]) . w2sum[e] = x . v[e] + c[e]
with v[e] = W1[e] @ w2sum[e]  (a [D] vector) and scalar
c[e] = b1[e].w2sum[e] + sum(b2[e]).  Then per token:
    s_e = x . v[e] + c[e],  logits = x @ Wg
    moe_sum = max(softmax(logits)) * s_argmax(logits)
    out = log_softmax over tokens (per batch row) of moe_sum.

Distribution over 8 cores, two launches (an on-device ncfw collective costs
~65us of barrier/trigger latency on this runtime, far more than a second
launch; the 16KB cross-core combine of v-partials happens on the host between
launches — the host does only that partial sum, all real math stays on device):
  launch A (expert-parallel over H): core c owns h-chunk [128c,128c+128) of
    both experts.  W2 ships bf16 d-major so w2sum is a PE ones-matmul
    (stationary [128d,128h] tiles, FWL bf16 loads), W1 ships bf16 h-major so
    v = w2sum^T-stationary @ W1-moving streams at 1 cyc/row.  Outputs
    [v0 | v1 | c0 c1] partials (16KB); host sums the 8 payloads.
  launch B (token-parallel): core c owns batch row c%4 (512 tokens).  x ships
    as a bf16 hi/lo pair (x = xh + xl exactly to ~2^-17), and one M=8
    stationary [wgh0 wgh1 wgl0 wgl1 vh0 vh1 vl0 vl1] (bf16 hi/lo of Wg and v)
    is streamed by xh then xl at 1 cyc/row: all four cross products accumulate
    in fp32 PSUM, so logits are fp32-grade (argmax must match the reference;
    bf16-only logits would flip near-boundary tokens) while the whole PE
    stream is 4x cheaper than an fp32 x stream.  l_e = col_e+col_{2+e},
    s_e = col_{4+e}+col_{6+e}+c_e after a PE transpose to token-major; then
    gate/select per token and the row log_softmax via PE transposes exactly
    as before (no cross-partition DMA).  Host takes rows from cores 0..3.

Scheduling: both launches issue the big HBM loads on the two HWDGE rings
(SP via nc.sync, ACT via nc.scalar) as their first instructions, before any
ACT-table load can head-of-line block a ring.  All hi/lo splits, transposes
and packing happen on the host (input reformatting only).
"""

import sys

import numpy as np

for _p in ("/opt/trn_rl_repo",):
    if _p not in sys.path:
        sys.path.append(_p)

import concourse.bass as bass  # noqa: E402
import concourse.mybir as mybir  # noqa: E402
import concourse.tile as tile  # noqa: E402
from concourse import bacc, bass_utils  # noqa: E402
from concourse.masks import make_identity  # noqa: E402

# Problem shape (hardcoded per spec).
B, T, D, H, E = 4, 512, 2048, 1024, 2
P = 128
NCORES = 8
TB = T  # tokens per core = one batch row
NB = D // P  # 16 d-blocks
HC = H // NCORES  # 128 h-chunk per expert per core
NG = TB // P  # 4 token groups per core
DC = D // NCORES  # 256 b2 columns per core
VK = 4  # v computed in VK chunks of D/VK columns
NSPLIT = 14  # xl d-blocks 0..NSPLIT-1 stream in launch A, the rest in B
NL = NSPLIT // 2  # each A core streams half its row's xl blocks (pair-split)
XB = NB + (NB - NSPLIT)  # moving blocks in launch B: xh 0..15 then xl NSPLIT..15
F32 = mybir.dt.float32
BF16 = mybir.dt.bfloat16
AX = mybir.AxisListType
AF = mybir.ActivationFunctionType
ALU = mybir.AluOpType

# launch A output: [128, E*NB + E] f32 — v partition-major (col e*NB+n on
# partition p holds v[e, n*128+p]) plus c0,c1 on partition 0
VCOLS = E * NB + E


def emit_phase_a(nc, tc, io):
    """w2sum (PE ones-matmul) + partial v for this core's H-chunk."""
    w2d, w1t, b1t, b2c = io["w2d"], io["w1t"], io["b1t"], io["b2c"]
    xlr, m4a, vout, lo_out = io["xlr"], io["m4a"], io["vout"], io["lo_out"]
    with (
        tc.tile_pool(name="main", bufs=1) as pool,
        tc.tile_pool(name="psum", bufs=1, space="PSUM") as psum,
    ):
        # Big loads first on both HWDGE rings, balanced ~1.75MB each.  W2
        # (d-major) gates the reduce so it leads ring 0; W1 per-expert leads
        # ring 1 so the v-chain starts early; the xl halves trail both rings.
        HS = 3  # xl blocks 0..2 ride ring 0; the rest ring 1
        w2_sb = pool.tile([P, NB, E, HC], BF16)
        w1_sb = pool.tile([P, E, D], BF16)
        xl_sb = pool.tile([P, NL, TB], BF16)
        HB = NB // 2
        nc.sync.dma_start(w2_sb[:, 0:HB], w2d[:, 0:HB])
        nc.scalar.dma_start(w1_sb[:, 0, :], w1t[:, 0, :])
        nc.sync.dma_start(w2_sb[:, HB:NB], w2d[:, HB:NB])
        nc.scalar.dma_start(w1_sb[:, 1, :], w1t[:, 1, :])
        nc.sync.dma_start(xl_sb[:, 0:HS], xlr[:, 0:HS])
        nc.scalar.dma_start(xl_sb[:, HS : NL - 1], xlr[:, HS : NL - 1])
        nc.scalar.dma_start(xl_sb[:, NL - 1 : NL], xlr[:, NL - 1 : NL])
        b1_sb = pool.tile([P, E], BF16)
        nc.gpsimd.dma_start(b1_sb[:], b1t)
        b2_sb = pool.tile([1, E * DC], F32)
        nc.gpsimd.dma_start(b2_sb[:], b2c)
        m4_sb = pool.tile([P, NL, 4], BF16)
        nc.gpsimd.dma_start(m4_sb[:], m4a[:])

        ones = pool.tile([P, 1], BF16)
        nc.vector.memset(ones[:], 1.0)

        # PE warm-up during the DMA window: sustained dummy matmuls ramp the
        # HAM clock so the real streams run fast (memsets on DVE, whose
        # queue frees up earliest)
        dum = pool.tile([P, 512], BF16)
        nc.vector.memset(dum[:], 0.25)
        wps = psum.tile([1, 512], F32, name="warmps")
        for _ in range(6):
            nc.tensor.matmul(wps[:], ones[:], dum[:], start=True, stop=True)

        # w2sum[e, h] via PE: stationary [128d, 128h] tiles, moving ones.
        w2ps = [psum.tile([P, 1], F32, name=f"w2ps_{e}") for e in range(E)]
        for n in range(NB):
            for e in range(E):
                nc.tensor.matmul(
                    w2ps[e][:],
                    w2_sb[:, n, e, :],
                    ones[:],
                    start=(n == 0),
                    stop=(n == NB - 1),
                )
        # first lo-correction blocks (chasing ring 0) keep the PE busy while
        # DVE builds w2hl below
        lo4 = psum.tile([4, TB], F32)
        for n in range(HS):
            nc.tensor.matmul(
                lo4[:], m4_sb[:, n, :], xl_sb[:, n, :], start=(n == 0), stop=False
            )

        # w2sum as a bf16 hi+lo column pair per expert (a single-bf16 cast
        # would dominate the accuracy budget)
        w2sf = pool.tile([P, E], F32)
        w2hl = pool.tile([P, E, 2], BF16)
        w2r32 = pool.tile([P, E], F32)
        for e in range(E):
            nc.vector.tensor_copy(w2sf[:, e : e + 1], w2ps[e][:])
            nc.vector.tensor_copy(w2hl[:, e, 0:1], w2ps[e][:])
        nc.vector.tensor_copy(w2r32[:], w2hl[:, :, 0])
        w2lo = pool.tile([P, E], F32)
        nc.vector.tensor_sub(w2lo[:], w2sf[:], w2r32[:])
        nc.vector.tensor_copy(w2hl[:, :, 1], w2lo[:])

        # v[e] = W1[e]^T-stationary @ [w2sum_hi | w2sum_lo]-moving: v comes
        # out PARTITION-major ([128, NB, 2] per expert), so the PSUM->SBUF
        # hop is two wide copies, not eight single-partition crawls
        pay3 = pool.tile([P, E, NB], F32)
        for e in range(E):
            vps = psum.tile([P, NB, 2], F32, name=f"vps_{e}")
            for n in range(NB):
                nc.tensor.matmul(
                    vps[:, n, :],
                    w1_sb[:, e, n * P : (n + 1) * P],
                    w2hl[:, e, :],
                    start=True,
                    stop=True,
                )
            vt = pool.tile([P, NB, 2], F32, name=f"vt_{e}")
            nc.vector.tensor_copy(vt[:], vps[:])
            nc.vector.tensor_add(
                pay3[:, e, :, None], vt[:, :, 0:1], vt[:, :, 1:2]
            )
        nc.sync.dma_start(vout[:, 0 : E * NB], pay3[:])

        # c[e] = b1[e].w2sum[e] + sum(b2[e])   (b1/b2 are zeros per spec,
        # kept for generality; bf16 b1 path is accuracy-irrelevant here)
        b1ps = psum.tile([1, E], F32)
        for e in range(E):
            nc.tensor.matmul(
                b1ps[0:1, e : e + 1],
                w2hl[:, e, 0:1],
                b1_sb[:, e : e + 1],
                start=True,
                stop=True,
            )
        b2s = pool.tile([1, E], F32)
        for e in range(E):
            nc.vector.reduce_sum(
                b2s[0:1, e : e + 1], b2_sb[0:1, e * DC : (e + 1) * DC], axis=AX.X
            )
        cpay = pool.tile([1, E], F32)
        nc.vector.tensor_add(cpay[:], b1ps[:], b2s[:])
        nc.gpsimd.dma_start(vout[0:1, E * NB : E * NB + E], cpay[:])

        # remaining exact xl @ [wgh|wgl] lo-correction blocks for this
        # core's half of its batch row (cores c and c+4 split the row's
        # blocks; the host sums the two partials and routes them to B)
        for n in range(HS, NL):
            nc.tensor.matmul(
                lo4[:],
                m4_sb[:, n, :],
                xl_sb[:, n, :],
                start=False,
                stop=(n == NL - 1),
            )
        lo_sb = pool.tile([4, TB], F32)
        nc.vector.tensor_copy(lo_sb[:], lo4[:])
        nc.scalar.dma_start(lo_out[:], lo_sb[:])


def emit_phase_b(nc, tc, io):
    """hi/lo bf16 logits+s stream, gate/select, row log_softmax."""
    x2, m8d, csum_d, lo8d, out = io["x2"], io["m8"], io["csum"], io["lo8"], io["out"]
    with (
        tc.tile_pool(name="main", bufs=1) as pool,
        tc.tile_pool(name="psum", bufs=1, space="PSUM") as psum,
    ):
        # m8 first (first matmul needs it), then the x blocks (xh 0..15,
        # then xl NSPLIT..15) alternating the two HWDGE rings; the last
        # chunk is kept small so the PE can finish right behind the DMA.
        m8 = pool.tile([P, NB, 8], BF16)
        nc.sync.dma_start(m8[:], m8d)
        x_sb = pool.tile([P, XB, TB], BF16)
        qs = [nc.sync, nc.scalar]
        chunks = [
            (1, 0, 1), (0, 1, 3),
            (1, 3, 6), (0, 6, 10),
            (1, 10, 14), (0, 14, 18),
            (1, 18, XB),
        ]
        for q, lo, hi in chunks:
            if lo < hi:
                qs[q].dma_start(x_sb[:, lo:hi], x2[:, lo:hi])
        csum = pool.tile([1, E], F32)
        nc.gpsimd.dma_start(csum[:], csum_d)
        lo8 = pool.tile([P, NG, E], F32)
        nc.gpsimd.dma_start(lo8[:], lo8d[:])

        # PE warm-up during the DMA window (HAM ramp; memsets on DVE whose
        # queue frees up earliest)
        dum = pool.tile([P, 512], BF16)
        nc.vector.memset(dum[:], 0.25)
        st1 = pool.tile([P, 1], BF16)
        nc.vector.memset(st1[:], 0.5)
        wps = psum.tile([1, 512], F32, name="warmps")
        for _ in range(6):
            nc.tensor.matmul(wps[:], st1[:], dum[:], start=True, stop=True)

        # preload the Exp table: the gate uses exp (sigmoid via 1/(1+e^-x))
        # so one table serves both the gate and the row softmax — no table
        # swap inside the tail (the cache holds ~one entry).  Reading csum
        # (not a const) delays this load until after the ring triggers, so
        # it can't head-of-line block the x DMA.
        wz = pool.tile([1, E], F32)
        nc.scalar.activation(wz[:], csum[0:1, :], AF.Exp)

        ident = pool.tile([P, P], F32)
        make_identity(nc, ident[:])
        # c broadcast tile on every partition, replicated per token group
        cb8 = pool.tile([P, NG, E], F32)
        for g in range(NG):
            nc.gpsimd.partition_broadcast(cb8[:, g, :], csum[0:1, :])

        # psum [8, TB] accumulates the xh stream (all blocks) and the tail
        # xl blocks against the M=8 stationary
        # [wgh0 wgh1 wgl0 wgl1 vh0 vh1 vl0 vl1] per d-block
        ps8 = psum.tile([8, TB], F32)
        for j in range(XB):
            n = j if j < NB else NSPLIT + (j - NB)
            nc.tensor.matmul(
                ps8[:],
                m8[:, n, :],
                x_sb[:, j, :],
                start=(j == 0),
                stop=(j == XB - 1),
            )
            if j in (0, 2, 5):
                # keep the PE busy across early chunk gaps so the HAM clock
                # doesn't re-throttle mid-stream
                nc.tensor.matmul(wps[:], st1[:], dum[:], start=True, stop=True)
        sbl = pool.tile([8, TB], F32)
        for g in range(NG):
            nc.vector.tensor_copy(
                sbl[0:8, g * P : (g + 1) * P], ps8[0:8, g * P : (g + 1) * P]
            )

        # token-major via 4 PE transposes into one PSUM tile, then ALL
        # gating math batched across the 4 groups in single strided DVE ops.
        # gate = softmax(l).max == sigmoid(|l0-l1|), mask = (l0 >= l1).
        tpa = psum.tile([P, NG, 8], F32)
        for g in range(NG):
            nc.tensor.transpose(
                tpa[:, g, :], sbl[0:8, g * P : (g + 1) * P], ident[0:8, 0:8]
            )
        t8a = pool.tile([P, NG, 8], F32)
        nc.vector.tensor_copy(t8a[:], tpa[:])
        l4 = pool.tile([P, NG, E], F32)
        nc.vector.tensor_add(l4[:], t8a[:, :, 0:2], t8a[:, :, 2:4])  # logits
        nc.vector.tensor_add(l4[:], l4[:], lo8[:])  # xl correction from A
        s4p = pool.tile([P, NG, E], F32)
        nc.vector.tensor_add(s4p[:], t8a[:, :, 4:6], t8a[:, :, 6:8])  # s
        nc.vector.tensor_add(s4p[:], s4p[:], cb8[:])
        dl = pool.tile([P, NG, 1], F32)
        nc.vector.tensor_sub(dl[:], l4[:, :, 0:1], l4[:, :, 1:2])
        ndl = pool.tile([P, NG, 1], F32)
        nc.vector.tensor_scalar_mul(ndl[:], dl[:], -1.0)
        nabs = pool.tile([P, NG, 1], F32)
        nc.vector.tensor_tensor(nabs[:], dl[:], ndl[:], op=ALU.min)
        egate = pool.tile([P, NG, 1], F32)
        nc.scalar.activation(egate[:], nabs[:], AF.Exp)
        den1 = pool.tile([P, NG, 1], F32)
        nc.vector.tensor_scalar_add(den1[:], egate[:], 1.0)
        gate = pool.tile([P, NG, 1], F32)
        nc.vector.reciprocal(gate[:], den1[:])
        mask = pool.tile([P, NG, 1], F32)
        nc.vector.tensor_scalar(mask[:], dl[:], 0.0, None, op0=ALU.is_ge)
        sdiff = pool.tile([P, NG, 1], F32)
        nc.vector.tensor_sub(sdiff[:], s4p[:, :, 0:1], s4p[:, :, 1:2])
        ssel = pool.tile([P, NG, 1], F32)
        nc.vector.tensor_mul(ssel[:], mask[:], sdiff[:])
        nc.vector.tensor_add(ssel[:], ssel[:], s4p[:, :, 1:2])
        moe_sb = pool.tile([P, NG], F32)
        nc.vector.tensor_mul(moe_sb[:, :, None], gate[:], ssel[:])

        # row log_softmax over all 512 tokens, via PE transposes
        tp4 = psum.tile([NG, P], F32)
        nc.tensor.transpose(tp4[:], moe_sb[:], ident[:])
        sb4t = pool.tile([NG, P], F32)
        nc.vector.tensor_copy(sb4t[:], tp4[:])
        m4p = pool.tile([NG, 1], F32)
        nc.vector.reduce_max(m4p[:], sb4t[:], axis=AX.X)
        m1p = psum.tile([1, NG], F32, name="m1p", tag="t1", bufs=2)
        nc.tensor.transpose(m1p[:], m4p[:], ident[0:NG, 0:NG])
        negm2 = pool.tile([1, 1], F32)
        nc.vector.reduce_max(negm2[:], m1p[:], axis=AX.X, negate=True)
        negm4 = pool.tile([NG, 1], F32)
        nc.gpsimd.partition_broadcast(negm4[:], negm2[:])
        e4 = pool.tile([NG, P], F32)
        s4 = pool.tile([NG, 1], F32)
        nc.scalar.activation(e4[:], sb4t[:], AF.Exp, bias=negm4[:], accum_out=s4[:])
        # load the Ln table NOW so the real Ln below table-hits; overlaps
        # the transpose+reduce running on other engines.  Input reads e4 to
        # pin this load after the row-Exp (scheduler ordering).
        wzl = pool.tile([1, 1], F32)
        nc.scalar.activation(wzl[:], e4[0:1, 0:1], AF.Ln)
        s1p = psum.tile([1, NG], F32, name="s1p", tag="t1", bufs=2)
        nc.tensor.transpose(s1p[:], s4[:], ident[0:NG, 0:NG])
        ssum = pool.tile([1, 1], F32)
        nc.vector.reduce_sum(ssum[:], s1p[:], axis=AX.X)
        logs = pool.tile([1, 1], F32)
        nc.scalar.activation(logs[:], ssum[:], AF.Ln)
        shift = pool.tile([1, 1], F32)
        nc.vector.tensor_sub(shift[:], negm2[:], logs[:])
        shift4 = pool.tile([NG, 1], F32)
        nc.gpsimd.partition_broadcast(shift4[:], shift[:])
        res4 = pool.tile([NG, P], F32)
        nc.vector.tensor_scalar_add(res4[:], sb4t[:], shift4[:])
        nc.sync.dma_start(out.rearrange("x (g p) -> g (x p)", p=P), res4[:])


_CACHED = {}


def build_program(which):
    if which in _CACHED:
        return _CACHED[which]
    nc = bacc.Bacc(
        "TRN2",
        target_bir_lowering=False,
        debug=False,
        enable_asserts=False,
        num_devices=NCORES,
    )
    if which == "a":
        io = {
            "w2d": nc.dram_tensor("w2d", [P, NB, E, HC], BF16, kind="ExternalInput").ap(),
            "w1t": nc.dram_tensor("w1t", [P, E, D], BF16, kind="ExternalInput").ap(),
            "b1t": nc.dram_tensor("b1t", [P, E], BF16, kind="ExternalInput").ap(),
            "b2c": nc.dram_tensor("b2c", [1, E * DC], F32, kind="ExternalInput").ap(),
            "xlr": nc.dram_tensor("xlr", [P, NL, TB], BF16, kind="ExternalInput").ap(),
            "m4a": nc.dram_tensor("m4a", [P, NL, 4], BF16, kind="ExternalInput").ap(),
            "vout": nc.dram_tensor("vout", [P, VCOLS], F32, kind="ExternalOutput").ap(),
            "lo_out": nc.dram_tensor("lo_out", [4, TB], F32, kind="ExternalOutput").ap(),
        }
        emit = emit_phase_a
    else:
        io = {
            "x2": nc.dram_tensor("x2", [P, XB, TB], BF16, kind="ExternalInput").ap(),
            "m8": nc.dram_tensor("m8", [P, NB, 8], BF16, kind="ExternalInput").ap(),
            "csum": nc.dram_tensor("csum", [1, E], F32, kind="ExternalInput").ap(),
            "lo8": nc.dram_tensor("lo8", [P, NG, E], F32, kind="ExternalInput").ap(),
            "out": nc.dram_tensor("out", [1, TB], F32, kind="ExternalOutput").ap(),
        }
        emit = emit_phase_b
    with tile.TileContext(nc) as tc:
        emit(nc, tc, io)
    nc.compile()
    _CACHED[which] = nc
    return nc


def _hi_lo(a):
    import ml_dtypes

    hi = a.astype(ml_dtypes.bfloat16)
    lo = (a - hi.astype(np.float32)).astype(ml_dtypes.bfloat16)
    return hi, lo


def shard_inputs_a(Wg, W1, b1, W2, b2, x):
    import ml_dtypes

    bf16 = ml_dtypes.bfloat16
    Wg = np.asarray(Wg, np.float32)
    W1 = np.asarray(W1, np.float32)
    b1 = np.asarray(b1, np.float32)
    W2 = np.asarray(W2, np.float32)
    b2 = np.asarray(b2, np.float32)
    x = np.asarray(x, np.float32).reshape(B * T, D)
    # m4a[p, n, :] = [wgh0 wgh1 wgl0 wgl1] at d = n*128 + p
    wgh, wgl = _hi_lo(Wg)  # [D, E]
    m4 = np.concatenate([wgh, wgl], axis=1)  # [D, 4]
    m4full = m4.reshape(NB, P, 4).transpose(1, 0, 2)  # [P, NB, 4]
    in_maps = []
    for c in range(NCORES):
        hs, he = c * HC, (c + 1) * HC
        # w2d[p, n, e, h] = W2[e, hs+h, p*16+n]  (d = p*16 + n: 8KB runs)
        w2d = np.ascontiguousarray(
            W2[:, hs:he, :].transpose(2, 0, 1).reshape(P, NB, E, HC).astype(bf16)
        )
        # w1t[h, e, d] = W1[e, d, hs+h]
        w1t = np.ascontiguousarray(W1[:, :, hs:he].transpose(2, 0, 1).astype(bf16))
        b1t = np.ascontiguousarray(b1[:, hs:he].T.astype(bf16))
        # xl residual of this core's batch row: cores c and c+4 split the
        # row's NSPLIT lo-blocks in half (host sums the two lo4 partials)
        row = c % B
        off = 0 if c < B else NL
        _, xl = _hi_lo(x[row * TB : (row + 1) * TB, :].T)  # [D, TB]
        xlr = np.ascontiguousarray(
            np.asarray(xl).reshape(NB, P, TB).transpose(1, 0, 2)[:, off : off + NL]
        )
        m4a = np.ascontiguousarray(m4full[:, off : off + NL])
        in_maps.append(
            {
                "w2d": w2d,
                "w1t": w1t,
                "b1t": b1t,
                "b2c": np.ascontiguousarray(
                    b2[:, c * DC : (c + 1) * DC].reshape(1, E * DC)
                ),
                "xlr": xlr,
                "m4a": m4a,
            }
        )
    return in_maps


def shard_inputs_b(x, Wg, vpart_sum, lo_rows):
    x = np.asarray(x, np.float32).reshape(B * T, D)
    Wg = np.asarray(Wg, np.float32)
    arr = np.asarray(vpart_sum, np.float32).reshape(P, VCOLS)
    vm = arr[:, : E * NB].reshape(P, E, NB)
    # v[e, n*128+p] = vm[p, e, n]
    v = np.stack([vm[:, e, :].T.reshape(-1) for e in range(E)])  # [E, D]
    csum = np.ascontiguousarray(arr[0:1, E * NB : E * NB + E])
    # m8[p, n, :] = [wgh0 wgh1 wgl0 wgl1 vh0 vh1 vl0 vl1] at d = n*128 + p
    wgh, wgl = _hi_lo(Wg)  # [D, E]
    vh, vl = _hi_lo(v.T)  # [D, E]
    m8 = np.concatenate([wgh, wgl, vh, vl], axis=1)  # [D, 8]
    m8 = np.ascontiguousarray(m8.reshape(NB, P, 8).transpose(1, 0, 2))
    # lo_rows[r] is launch A's [4, TB] xl@[wgh|wgl] partial for batch row r;
    # pairwise row-sum -> per-token logit correction, token-major [P, NG, E]
    lo_rows = np.asarray(lo_rows, np.float32)  # [B, 4, TB]
    lo = lo_rows[:, 0:2, :] + lo_rows[:, 2:4, :]  # [B, E, TB]
    in_maps = []
    for c in range(NCORES):
        row = c % B
        xr = x[row * TB : (row + 1) * TB, :]  # [TB, D]
        xh, xl = _hi_lo(xr.T)  # [D, TB]
        xh3 = np.asarray(xh).reshape(NB, P, TB)
        xl3 = np.asarray(xl).reshape(NB, P, TB)
        # x2[p, j, t]: xh blocks 0..15 then xl blocks NSPLIT..15
        x2 = np.ascontiguousarray(
            np.concatenate([xh3, xl3[NSPLIT:]], axis=0).transpose(1, 0, 2)
        )
        lo8 = np.ascontiguousarray(
            lo[row].T.reshape(NG, P, E).transpose(1, 0, 2)
        )  # lo8[p, g, e] = lo[row, e, g*128+p]
        in_maps.append({"x2": x2, "m8": m8, "csum": csum, "lo8": lo8})
    return in_maps


def run_a(in_maps, **kwargs):
    return bass_utils.run_bass_kernel_spmd(
        build_program("a"), in_maps, core_ids=list(range(NCORES)), **kwargs
    )


def run_b(in_maps, **kwargs):
    return bass_utils.run_bass_kernel_spmd(
        build_program("b"), in_maps, core_ids=list(range(NCORES)), **kwargs
    )


def kernel(x, Wg, W1, b1, W2, b2):
    res_a = run_a(shard_inputs_a(Wg, W1, b1, W2, b2, x))
    # cross-core combine: sum of the 8 per-core v/c partials and gather of
    # the per-row xl logit partials (the reshard step between the launches;
    # ~24KB, no model math beyond the partial-sum reductions)
    vpart = np.sum([res_a.results[c]["vout"] for c in range(NCORES)], axis=0)
    vpart = np.ascontiguousarray(vpart, np.float32)
    lo_rows = np.stack(
        [res_a.results[r]["lo_out"] + res_a.results[r + B]["lo_out"] for r in range(B)]
    )
    res_b = run_b(shard_inputs_b(x, Wg, vpart, lo_rows))
    return np.concatenate([res_b.results[b]["out"] for b in range(B)], axis=0)
